# revision 39
# baseline (speedup 1.0000x reference)
"""Trainium2 Bass/Tile kernel for a dense transformer block (B=2, T=2048, D=1024, H=16).

Sharding across 8 NeuronCores (head-parallel attention + token-parallel FFN):
  - LN1 statistics: each core computes stats for its own 512 tokens, then a
    tiny AllGather ([3,512] -> [24,512]) replicates (-mu, sd, 1/sd) rows.
  - LN1 *apply* is folded into the PE: raw QKV psums get one extra accumulated
    "fix" matmul (wsums^T @ [-mu; sd]); Q is then scaled by the broadcast A row
    (one DVE op); K stays unscaled and its per-key scale A_k is applied as the
    per-partition `scale` of the exp activation; V stays unscaled and A_k is
    folded into the per-key 1/c normalizer.
  - Attention is head-sharded (2 heads/core, both batches). Query-axis softmax:
      attn^T = (V*A/c)^T @ exp(A_k * (S^T + mask_bias)),  c[k] = row-sum
    with the causal mask applied as an additive -30 bias via one PE matmul,
    so each (key-tile, head) needs exactly ONE exp activation (+accum for c).
  - One 2MB/core AllToAll per batch reshards attention output to token-split.
  - Projection + residual, LN2 and the FFN run token-sharded; the LN2 apply,
    b1/b2 biases and the LN2 gamma are all folded into W1/W2 fix matmuls on
    the PE (relu(A2*x) = A2*relu(x) since A2>0), with the A2 column scale
    applied once at the output stage.
Everything is feature-major ("transposed") so every matmul has a natural lhsT.
"""

import numpy as np
import ml_dtypes

import concourse.bass as bass
import concourse.bacc as bacc
import concourse.mybir as mybir
import concourse.tile as tile
from concourse.bass_utils import run_bass_kernel_spmd

F32 = mybir.dt.float32
BF16 = mybir.dt.bfloat16
AF = mybir.ActivationFunctionType
ALU = mybir.AluOpType

B, T, D, H = 2, 2048, 1024, 16
HS = D // H          # 64
DFF = 4 * D          # 4096
EPS = 1e-5
NC_ = 8              # cores
BT = B * T           # 4096 flat tokens
TOK = BT // NC_      # 512 tokens per core
NBLK = BT // 512     # 8 token blocks
NDC = D // 128       # 8 d-chunks
NH_LOC = H // NC_    # 2 heads per core
NKT = T // 128       # 16 key tiles per batch
NHT = DFF // 128     # 32 hidden tiles
MASKB = -30.0        # additive pre-exp mask bias


def _build_nc():
    nc = bacc.Bacc(num_devices=NC_)

    xt_d = nc.dram_tensor("xt", [NBLK, 128, NDC, 512], BF16, kind="ExternalInput")
    xtloc_d = nc.dram_tensor("xt_loc", [128, NDC, 512], F32, kind="ExternalInput")
    wqkv_d = nc.dram_tensor("wqkv", [D, 384], BF16, kind="ExternalInput")
    wsb2_d = nc.dram_tensor("wsb2", [2, 384], F32, kind="ExternalInput")
    wsr_d = nc.dram_tensor("ws_row", [1, 384], F32, kind="ExternalInput")
    wbr_d = nc.dram_tensor("wb_row", [1, 384], F32, kind="ExternalInput")
    wproj_d = nc.dram_tensor("wproj", [128, NDC, D], BF16, kind="ExternalInput")
    w1_d = nc.dram_tensor("w1", [NHT, 128, NDC, 128], BF16, kind="ExternalInput")
    w1fix_d = nc.dram_tensor("w1fix", [2, DFF], F32, kind="ExternalInput")
    w2_d = nc.dram_tensor("w2", [NDC, 128, NHT, 128], BF16, kind="ExternalInput")
    bproj_d = nc.dram_tensor("bproj_pp", [128, 8], F32, kind="ExternalInput")
    b2p_d = nc.dram_tensor("b2p_pp", [128, 8], F32, kind="ExternalInput")
    b1ln_d = nc.dram_tensor("b1ln_pp", [128, 8], F32, kind="ExternalInput")
    g1diag_d = nc.dram_tensor("g1diag", [128, NDC, 128], F32, kind="ExternalInput")
    g1row_d = nc.dram_tensor("g1row", [1, D], F32, kind="ExternalInput")
    g2diag_d = nc.dram_tensor("g2diag", [128, NDC, 128], BF16, kind="ExternalInput")
    g2row_d = nc.dram_tensor("g2row", [1, D], F32, kind="ExternalInput")
    maskb_d = nc.dram_tensor("maskb", [128, 128], BF16, kind="ExternalInput")
    identb_d = nc.dram_tensor("identb", [128, 128], BF16, kind="ExternalInput")
    ident_d = nc.dram_tensor("ident", [128, 128], F32, kind="ExternalInput")
    onesd_d = nc.dram_tensor("ones_d", [128, 1], F32, kind="ExternalInput")    # 1/D
    onesdb_d = nc.dram_tensor("ones_db", [128, 1], BF16, kind="ExternalInput")  # 1/D bf16
    ones1_d = nc.dram_tensor("ones_1", [1, 128], F32, kind="ExternalInput")
    ones1b_d = nc.dram_tensor("ones_1b", [1, 128], BF16, kind="ExternalInput")
    out_d = nc.dram_tensor("outT", [D, TOK], F32, kind="ExternalOutput")

    with tile.TileContext(nc) as tc:
        with tc.tile_pool(name="const", bufs=1) as cst, \
             tc.tile_pool(name="dram", bufs=1, space="DRAM") as dpool:
            def cload(shape, dram_ap, dtype=F32):
                t = cst.tile(shape, dtype, name=f"c{len(nc.m.functions[0].allocations)}")
                nc.gpsimd.dma_start(t[:], dram_ap)
                return t

            # dummy collective FIRST: absorbs the ~30us first-collective
            # firmware barrier while real work proceeds (result unused)
            dum_in = dpool.tile([1, 8], BF16, name="dum_i")
            dum_out = dpool.tile([8, 8], BF16, name="dum_o")

            # ---- HAM warmup: keep PE busy while constants stream in ----
            with tc.tile_pool(name="warm", bufs=1) as wp, \
                 tc.tile_pool(name="warm_ps", bufs=1, space="PSUM") as wps:
                wt_ = wp.tile([128, 512], BF16)
                nc.vector.memset(wt_[:], 0.001)
                nc.sync.dma_start(dum_in[:], wt_[0:1, 0:8])
                nc.gpsimd.collective_compute(
                    "AllGather", ALU.bypass,
                    replica_groups=[list(range(NC_))],
                    ins=[dum_in.opt()], outs=[dum_out.opt()])
                wpt = wps.tile([128, 512], F32)
                for _ in range(16):
                    nc.tensor.matmul(wpt[:], wt_[:, 0:128], wt_[:], start=True, stop=True)

            wqkv_sb = cload([128, NDC, 384], wqkv_d[:, :].rearrange("(a p) m -> p a m", p=128), BF16)
            maskb_sb = cload([128, 128], maskb_d[:, :], BF16)
            identb_sb = cload([128, 128], identb_d[:, :], BF16)
            ident_sb = cload([128, 128], ident_d[:, :])
            onesd_sb = cload([128, 1], onesd_d[:, :])
            onesdb_sb = cload([128, 1], onesdb_d[:, :], BF16)
            ones1_sb = cload([1, 128], ones1_d[:, :])
            wsb2_sb = cload([2, 384], wsb2_d[:, :])
            wsr_sb = cload([1, 384], wsr_d[:, :])
            wbr_sb = cload([1, 384], wbr_d[:, :])
            ones1b_sb = cload([1, 128], ones1b_d[:, :], BF16)
            g1diag_sb = cload([128, NDC, 128], g1diag_d[:, :, :])
            g1row_sb = cload([1, D], g1row_d[:, :])
            g2diag_sb = cload([128, NDC, 128], g2diag_d[:, :, :], BF16)
            g2row_sb = cload([1, D], g2row_d[:, :])
            bproj_sb = cload([128, 8], bproj_d[:, :])
            b2p_sb = cload([128, 8], b2p_d[:, :])
            b1ln_sb = cload([128, 8], b1ln_d[:, :])
            w1fix_sb = cload([2, DFF], w1fix_d[:, :])

            xw = cst.tile([128, NDC, 512], F32)      # x_loc -> xn_loc -> x1_loc
            x1b = cst.tile([128, NDC, 512], BF16)    # bf16 copy of x1 for FFN
            stats2 = cst.tile([2, 512], F32)         # LN2 rows (-mu2; sd2)
            a2bc = cst.tile([128, 512], F32)         # broadcast A2 row
            Gr_sb = cst.tile([2, NC_, 512], F32)     # gathered (-mu, sd) rows per rank
            Ar_sb = cst.tile([1, NC_, 512], F32)     # gathered 1/sd rows per rank
            GT_sb = cst.tile([128, 4, 24], F32)      # transposed stats (A_k columns)
            hT = cst.tile([128, NHT, 512], BF16)     # FFN hidden
            attn_loc = cst.tile([128, NDC, 512], BF16)

            nc.sync.dma_start(xw[:], xtloc_d[:, :, :])

            # ===== local LN1 stats for my 512 tokens + AllGather =====
            with (
                tc.tile_pool(name="st_sm", bufs=1) as stp,
                tc.tile_pool(name="st_ps", bufs=1, space="PSUM") as stps,
            ):
                s1p = stps.tile([1, 512], F32)
                s2p = stps.tile([1, 512], F32)
                for a in range(NDC):
                    sq = stp.tile([128, 512], BF16, tag="sq")
                    if a % 2 == 0:
                        nc.scalar.square(sq[:], xw[:, a, :])
                    else:
                        nc.vector.tensor_tensor(sq[:], xw[:, a, :], xw[:, a, :], ALU.mult)
                    nc.tensor.matmul(s1p[:], onesd_sb[:], xw[:, a, :],
                                     start=(a == 0), stop=(a == NDC - 1))
                    nc.tensor.matmul(s2p[:], onesdb_sb[:], sq[:],
                                     start=(a == 0), stop=(a == NDC - 1))
                nmu_l = stp.tile([1, 512], F32)   # -mu
                sd_l = stp.tile([1, 512], F32)    # sd
                arow_l = stp.tile([1, 512], F32)  # 1/sd
                msq = stp.tile([1, 512], F32)
                nc.scalar.square(msq[:], s1p[:])
                veps = stp.tile([1, 512], F32)
                nc.vector.scalar_tensor_tensor(veps[:], s2p[:], EPS, msq[:],
                                               ALU.add, ALU.subtract)
                nc.scalar.sqrt(sd_l[:], veps[:])
                nc.vector.reciprocal(arow_l[:], sd_l[:])
                nc.vector.tensor_scalar_mul(nmu_l[:], s1p[:], -1.0)

                ag_in = dpool.tile([3, 512], F32, name="ag_in")
                ag_out = dpool.tile([24, 512], F32, name="ag_out")
                nc.sync.dma_start(ag_in[0:1], nmu_l[:])
                nc.sync.dma_start(ag_in[1:2], sd_l[:])
                nc.sync.dma_start(ag_in[2:3], arow_l[:])
                nc.gpsimd.collective_compute(
                    "AllGather", ALU.bypass,
                    replica_groups=[list(range(NC_))],
                    ins=[ag_in.opt()], outs=[ag_out.opt()])
                ag_r = ag_out[:, :].rearrange("(r s) n -> s r n", s=3)
                nc.sync.dma_start(Gr_sb[:], ag_r[0:2])
                nc.sync.dma_start(Ar_sb[:], ag_r[2:3])
                G24 = stp.tile([24, 512], F32)
                nc.sync.dma_start(G24[:], ag_out[:, :])
                for c4 in range(4):
                    gtp = stps.tile([128, 24], F32, tag="gtp", bufs=2)
                    nc.tensor.transpose(gtp[:], G24[:, 128 * c4:128 * (c4 + 1)],
                                        ident_sb[0:24, 0:24])
                    nc.vector.tensor_copy(GT_sb[:, c4, :], gtp[:])

                # ---- LN1 apply on local slice: xn = Abc*(g1*x + g1*(-mu)) + b1ln ----
                abl = stps.tile([128, 512], F32)
                nc.tensor.matmul(abl[:], ones1_sb[:], arow_l[:], start=True, stop=True)
                abl_sb = stp.tile([128, 512], F32)
                nc.scalar.copy(abl_sb[:], abl[:])
                for a in range(NDC):
                    pa = stps.tile([128, 512], F32, tag="pa", bufs=2)
                    nc.tensor.matmul(pa[:], g1diag_sb[:, a, :], xw[:, a, :],
                                     start=True, stop=False)
                    nc.tensor.matmul(pa[:], g1row_sb[:, 128 * a:128 * (a + 1)],
                                     nmu_l[:], start=False, stop=True)
                    t1 = stp.tile([128, 512], F32, tag="t1", bufs=2)
                    nc.vector.tensor_tensor(t1[:], pa[:], abl_sb[:], ALU.mult)
                    nc.scalar.activation(xw[:, a, :], t1[:], AF.Identity,
                                         bias=b1ln_sb[:, a:a + 1])

            # ===== big attention scope =====
            with tc.tile_pool(name="attn_scope", bufs=1) as big:
                Q2 = big.tile([128, BT], BF16)
                K2 = big.tile([128, BT], BF16)
                VK = big.tile([128, BT], BF16)   # k-major V, heads side by side
                attnT = big.tile([128, BT], BF16)

                # ===== Phase A: raw QKV + PE-folded LN fixups =====
                with (
                    tc.tile_pool(name="sweep", bufs=2) as swp,
                    tc.tile_pool(name="sweep_ps", bufs=2, space="PSUM") as swps,
                    tc.tile_pool(name="fix_ps", bufs=1, space="PSUM") as fxps,
                ):
                    NLOC = 5   # blocks fixed from locally-computed stats (no gather wait)
                    for tb in range(NBLK):
                        xblk = swp.tile([128, NDC, 512], BF16, tag="xblk")
                        nc.sync.dma_start(xblk[:], xt_d[tb])
                        # rank/col-offset of this block's two 256-token halves
                        halves = [(2 * (tb % 4) + h, 256 * (tb // 4)) for h in range(2)]
                        if tb < NLOC:
                            # local stats for this block's 512 tokens (bf16 source)
                            s1b = fxps.tile([1, 512], F32, tag="s1b")
                            s2b = fxps.tile([1, 512], F32, tag="s2b")
                            for a in range(NDC):
                                sq = swp.tile([128, 512], BF16, tag="sqb", bufs=1)
                                if a % 2 == 0:
                                    nc.scalar.square(sq[:], xblk[:, a, :])
                                else:
                                    nc.vector.tensor_tensor(sq[:], xblk[:, a, :],
                                                            xblk[:, a, :], ALU.mult)
                                nc.tensor.matmul(s1b[:], onesdb_sb[:], xblk[:, a, :],
                                                 start=(a == 0), stop=(a == NDC - 1))
                                nc.tensor.matmul(s2b[:], onesdb_sb[:], sq[:],
                                                 start=(a == 0), stop=(a == NDC - 1))
                            msq = swp.tile([1, 512], F32, tag="msqb", bufs=1)
                            nc.scalar.square(msq[:], s1b[:])
                            veps = swp.tile([1, 512], F32, tag="vepsb", bufs=1)
                            nc.vector.scalar_tensor_tensor(veps[:], s2b[:], EPS, msq[:],
                                                           ALU.add, ALU.subtract)
                            sd_tb = swp.tile([1, 512], F32, tag="sdb", bufs=1)
                            ar_tb = swp.tile([1, 512], F32, tag="arb", bufs=1)
                            nmu_tb = swp.tile([1, 512], F32, tag="nmub", bufs=1)
                            nc.scalar.sqrt(sd_tb[:], veps[:])
                            nc.vector.reciprocal(ar_tb[:], sd_tb[:])
                            nc.vector.tensor_scalar_mul(nmu_tb[:], s1b[:], -1.0)
                        qkps = []
                        for j in range(3):
                            ps = swps.tile([128, 512], F32, tag="qkvps", bufs=3)
                            for a in range(NDC):
                                nc.tensor.matmul(ps[:], wqkv_sb[:, a, 128 * j:128 * (j + 1)],
                                                 xblk[:, a, :],
                                                 start=(a == 0),
                                                 stop=(tb >= NLOC and a == NDC - 1))
                            if tb < NLOC:
                                nc.tensor.matmul(ps[:], wsr_sb[:, 128 * j:128 * (j + 1)],
                                                 nmu_tb[:], start=False, stop=False)
                                nc.tensor.matmul(ps[:], wbr_sb[:, 128 * j:128 * (j + 1)],
                                                 sd_tb[:], start=False, stop=True)
                            else:
                                for h, (r, c0) in enumerate(halves):
                                    nc.tensor.matmul(
                                        ps[:, 256 * h:256 * (h + 1)],
                                        wsb2_sb[:, 128 * j:128 * (j + 1)],
                                        Gr_sb[:, r, c0:c0 + 256],
                                        start=False, stop=(h == 1), skip_group_check=True)
                            qkps.append(ps)
                        bap = fxps.tile([128, 512], F32, tag="bap")
                        if tb < NLOC:
                            nc.tensor.matmul(bap[:], ones1_sb[:], ar_tb[:],
                                             start=True, stop=True)
                        else:
                            for h, (r, c0) in enumerate(halves):
                                nc.tensor.matmul(bap[:, 256 * h:256 * (h + 1)], ones1_sb[:],
                                                 Ar_sb[:, r, c0:c0 + 256],
                                                 start=(h == 0), stop=(h == 1),
                                                 skip_group_check=True)
                        ba_sb = swp.tile([128, 512], F32, tag="ba_sb")
                        nc.scalar.copy(ba_sb[:], bap[:])
                        vsb = swp.tile([128, 512], F32, tag="vsb", bufs=1)
                        nc.scalar.copy(vsb[:], qkps[2][:])
                        ts_ = slice(512 * tb, 512 * (tb + 1))
                        nc.scalar.copy(K2[:, ts_], qkps[1][:])
                        nc.vector.tensor_tensor(Q2[:, ts_], qkps[0][:], ba_sb[:], ALU.mult)
                        for q in range(4):
                            vtp = fxps.tile([128, 128], F32, tag="vtp", bufs=2)
                            nc.tensor.transpose(vtp[:], vsb[:, 128 * q:128 * (q + 1)],
                                                ident_sb[:])
                            nc.vector.tensor_copy(
                                VK[:, 512 * tb + 128 * q:512 * tb + 128 * (q + 1)],
                                vtp[:])

                # ===== Phase B: attention per (batch, key-tile), A2A per batch =====
                with (
                    tc.tile_pool(name="se_pool", bufs=2) as sep,
                    tc.tile_pool(name="attn_sm", bufs=2) as asm,
                    tc.tile_pool(name="st_ps2", bufs=1, space="PSUM") as sps2,
                    tc.tile_pool(name="ap_ps", bufs=1, space="PSUM") as apps,
                ):
                    for b in range(B):
                        ap_tiles = [apps.tile([128, 512], F32, name=f"app{b}_{qb}",
                                              tag=f"ap{qb}", bufs=1)
                                    for qb in range(4)]
                        se_tiles, vc_tiles, st_tiles, ck_tiles = {}, {}, {}, {}

                        def ascale(kt):
                            return GT_sb[:, 2 * b + (kt % 2), 3 * (kt // 2) + 2:
                                         3 * (kt // 2) + 3]

                        def st_mms(kt, hh, c):
                            # chunk c covers region cols [1024c, min(1024(c+1), W))
                            qb0 = kt // 4
                            co = 128 * (kt % 4)
                            W = 2048 - 512 * qb0
                            q0 = b * T + 512 * qb0
                            stp_ = sps2.tile([128, 1024], F32, tag=f"st{hh}", bufs=1)
                            st_tiles[(kt, hh, c)] = stp_
                            hsl = slice(64 * hh, 64 * hh + 64)
                            lo = max(co, 1024 * c)
                            hi = min(W, 1024 * (c + 1))
                            g = lo
                            while g < hi:
                                g2 = min(hi, (g // 512 + 1) * 512)
                                nc.tensor.matmul(
                                    stp_[:, g - 1024 * c:g2 - 1024 * c],
                                    K2[hsl, b * T + 128 * kt:b * T + 128 * (kt + 1)],
                                    Q2[hsl, q0 + g:q0 + g2],
                                    start=True, stop=False,
                                    skip_group_check=True)
                                g = g2
                            if c == 0:
                                nc.tensor.matmul(
                                    stp_[:, co:co + 128], identb_sb[:],
                                    maskb_sb[:, :],
                                    start=False, stop=True, skip_group_check=True)

                        def exp_chunk(kt, hh, c, nchunk):
                            qb0 = kt // 4
                            co = 128 * (kt % 4)
                            W = 2048 - 512 * qb0
                            stp_ = st_tiles.pop((kt, hh, c))
                            lo = max(co, 1024 * c)
                            hi = min(W, 1024 * (c + 1))
                            if c == 0:
                                se = sep.tile([128, 2048], BF16,
                                              name=f"se_{b}_{kt}_{hh}",
                                              tag=f"se{hh}", bufs=3)
                                se_tiles[(kt, hh)] = se
                                ck = asm.tile([128, 2], F32, tag=f"ck{hh}", bufs=2)
                                ck_tiles[(kt, hh)] = ck
                            se = se_tiles[(kt, hh)]
                            ck = ck_tiles[(kt, hh)]
                            nc.scalar.activation(
                                se[:, lo:hi], stp_[:, lo - 1024 * c:hi - 1024 * c],
                                AF.Exp, scale=ascale(kt),
                                accum_out=ck[:, c:c + 1])
                            if c == nchunk - 1:
                                if nchunk == 2:
                                    cks = asm.tile([128, 1], F32, tag=f"cks{hh}", bufs=2)
                                    nc.vector.tensor_tensor(cks[:], ck[:, 0:1],
                                                            ck[:, 1:2], ALU.add)
                                    cks_ap = cks[:]
                                else:
                                    cks_ap = ck[:, 0:1]
                                rk = asm.tile([128, 1], F32, tag=f"rk{hh}", bufs=2)
                                nc.vector.reciprocal(rk[:], cks_ap)
                                ark = asm.tile([128, 1], F32, tag=f"ark{hh}", bufs=2)
                                nc.vector.tensor_tensor(ark[:], rk[:], ascale(kt),
                                                        ALU.mult)
                                vc = asm.tile([128, 64], BF16, tag=f"vc{hh}", bufs=2)
                                ktf = b * NKT + kt
                                nc.vector.tensor_scalar_mul(
                                    vc[:],
                                    VK[:, 128 * ktf + 64 * hh:128 * ktf + 64 * hh + 64],
                                    ark[:])
                                vc_tiles[(kt, hh)] = vc

                        def emit_at(kt):
                            qb0 = kt // 4
                            co = 128 * (kt % 4)
                            for qb in range(qb0, 4):
                                off = 512 * (qb - qb0)
                                n0 = co if qb == qb0 else 0
                                last = (kt == 4 * (qb + 1) - 1)
                                for hh in range(NH_LOC):
                                    nc.tensor.matmul(
                                        ap_tiles[qb][64 * hh:64 * hh + 64, n0:512],
                                        vc_tiles[(kt, hh)][:],
                                        se_tiles[(kt, hh)][:, off + n0:off + 512],
                                        start=(kt == 0), stop=last,
                                        tile_position=(0, 64 * hh),
                                        skip_group_check=True)
                                if last:
                                    nc.vector.tensor_copy(
                                        attnT[:, b * T + 512 * qb:b * T + 512 * (qb + 1)],
                                        ap_tiles[qb][:])

                        for kt in range(NKT + 1):
                            if kt < NKT:
                                W = 2048 - 512 * (kt // 4)
                                nchunk = (W + 1023) // 1024
                                for c in range(nchunk):
                                    st_mms(kt, 0, c)
                                    st_mms(kt, 1, c)
                                    exp_chunk(kt, 0, c, nchunk)
                                    exp_chunk(kt, 1, c, nchunk)
                            if kt >= 1:
                                emit_at(kt - 1)

                        a2a_in = dpool.tile([8, 128, 256], BF16, name=f"a2ai{b}", tag=f"a2ai{b}")
                        a2a_out = dpool.tile([8, 128, 256], BF16, name=f"a2ao{b}", tag=f"a2ao{b}")
                        for j in range(8):
                            nc.sync.dma_start(a2a_in[j],
                                              attnT[:, b * T + 256 * j:b * T + 256 * (j + 1)])
                        nc.gpsimd.collective_compute(
                            "AllToAll", ALU.bypass,
                            replica_groups=[list(range(NC_))],
                            ins=[a2a_in.opt()], outs=[a2a_out.opt()])
                        for s in range(8):
                            nc.sync.dma_start(attn_loc[:, s, 256 * b:256 * (b + 1)], a2a_out[s])

            # ===== Phase D: projection + residual + LN2 stats (PE-folded apply) =====
            with (
                tc.tile_pool(name="proj_sm", bufs=2) as pjm,
                tc.tile_pool(name="proj_ps", bufs=2, space="PSUM") as pjps,
            ):
                for dt in range(NDC):
                    wpt = pjm.tile([128, NDC, 128], BF16, tag="wp", bufs=3)
                    nc.sync.dma_start(wpt[:], wproj_d[:, :, 128 * dt:128 * (dt + 1)])
                    pp = pjps.tile([128, 512], F32, tag="pp")
                    for a in range(NDC):
                        nc.tensor.matmul(pp[:], wpt[:, a, :],
                                         attn_loc[:, a, :],
                                         start=(a == 0), stop=(a == NDC - 1))
                    nc.vector.scalar_tensor_tensor(
                        xw[:, dt, :], pp[:], bproj_sb[:, dt:dt + 1], xw[:, dt, :],
                        ALU.add, ALU.add)
                    nc.scalar.copy(x1b[:, dt, :], xw[:, dt, :])
                # LN2 stats on x1
                s1p = pjps.tile([1, 512], F32, tag="s1b", bufs=1)
                s2p = pjps.tile([1, 512], F32, tag="s2b", bufs=1)
                for a in range(NDC):
                    sq = pjm.tile([128, 512], BF16, tag="sq2")
                    nc.vector.tensor_tensor(sq[:], x1b[:, a, :], x1b[:, a, :], ALU.mult)
                    nc.tensor.matmul(s1p[:], onesd_sb[:], xw[:, a, :],
                                     start=(a == 0), stop=(a == NDC - 1))
                    nc.tensor.matmul(s2p[:], onesdb_sb[:], sq[:],
                                     start=(a == 0), stop=(a == NDC - 1))
                msq = pjm.tile([1, 512], F32, tag="msq2", bufs=1)
                nc.scalar.square(msq[:], s1p[:])
                veps = pjm.tile([1, 512], F32, tag="veps2", bufs=1)
                nc.vector.scalar_tensor_tensor(veps[:], s2p[:], EPS, msq[:],
                                               ALU.add, ALU.subtract)
                sd2_l = pjm.tile([1, 512], F32, tag="sd2l", bufs=1)
                nc.scalar.sqrt(sd2_l[:], veps[:])
                a2row = pjm.tile([1, 512], F32, tag="a2row", bufs=1)
                nc.vector.reciprocal(a2row[:], sd2_l[:])
                nc.vector.tensor_scalar_mul(stats2[0:1, :], s1p[:], -1.0)
                nc.sync.dma_start(stats2[1:2, :], sd2_l[:])
                a2p = pjps.tile([128, 512], F32, tag="a2p", bufs=1)
                nc.tensor.matmul(a2p[:], ones1_sb[:], a2row[:], start=True, stop=True)
                nc.scalar.copy(a2bc[:], a2p[:])

            # ===== Phase E: FFN (token-sharded, streamed weights) =====
            with (
                tc.tile_pool(name="ffn_w1", bufs=3) as w1p,
                tc.tile_pool(name="ffn_w2", bufs=2) as w2p,
                tc.tile_pool(name="ffn_sm", bufs=2) as fsm,
                tc.tile_pool(name="ffn_ps", bufs=2, space="PSUM") as fps,
            ):
                for ht in range(NHT):
                    w1t = w1p.tile([128, NDC, 128], BF16, tag="w1", bufs=4)
                    nc.sync.dma_start(w1t[:], w1_d[ht])
                    hp = fps.tile([128, 512], F32, tag="hp")
                    for a in range(NDC):
                        nc.tensor.matmul(hp[:], w1t[:, a, :], x1b[:, a, :],
                                         start=(a == 0), stop=False)
                    nc.tensor.matmul(hp[:], w1fix_sb[:, 128 * ht:128 * (ht + 1)],
                                     stats2[:, :], start=False, stop=True)
                    nc.scalar.activation(hT[:, ht, :], hp[:], AF.Relu)
                for dt in range(NDC):
                    w2t = w2p.tile([128, NHT, 128], BF16, tag="w2")
                    nc.sync.dma_start(w2t[:], w2_d[dt])
                    fp_ = fps.tile([128, 512], F32, tag="fp")
                    for a2_ in range(NHT):
                        nc.tensor.matmul(fp_[:], w2t[:, a2_, :], hT[:, a2_, :],
                                         start=(a2_ == 0), stop=False)
                    nc.tensor.matmul(fp_[:], g2diag_sb[:, dt, :], x1b[:, dt, :],
                                     start=False, stop=False)
                    nc.tensor.matmul(fp_[:], g2row_sb[:, 128 * dt:128 * (dt + 1)],
                                     stats2[0:1, :], start=False, stop=True)
                    t1 = fsm.tile([128, 512], F32, tag="t1f")
                    nc.vector.tensor_tensor(t1[:], fp_[:], a2bc[:], ALU.mult)
                    ot = fsm.tile([128, 512], F32, tag="ot")
                    nc.scalar.activation(ot[:], t1[:], AF.Identity,
                                         bias=b2p_sb[:, dt:dt + 1])
                    nc.sync.dma_start(
                        out_d[:, :].rearrange("(a p) n -> p a n", p=128)[:, dt, :], ot[:])
    nc.compile()
    return nc


_NC_CACHE = None


def _get_nc():
    global _NC_CACHE
    if _NC_CACHE is None:
        _NC_CACHE = _build_nc()
    return _NC_CACHE


def make_in_maps(inputs):
    x = np.asarray(inputs["x"], np.float32)
    Wq = np.asarray(inputs["Wq"], np.float32)
    Wk = np.asarray(inputs["Wk"], np.float32)
    Wv = np.asarray(inputs["Wv"], np.float32)
    Wproj = np.ascontiguousarray(np.asarray(inputs["Wproj"], np.float32))
    bproj = np.asarray(inputs["bproj"], np.float32)
    W1 = np.ascontiguousarray(np.asarray(inputs["W1"], np.float32))
    b1 = np.asarray(inputs["b1"], np.float32)
    W2 = np.ascontiguousarray(np.asarray(inputs["W2"], np.float32))
    b2 = np.asarray(inputs["b2"], np.float32)
    g1 = np.asarray(inputs["ln1_g"], np.float32)
    bl1 = np.asarray(inputs["ln1_b"], np.float32)
    g2 = np.asarray(inputs["ln2_g"], np.float32)
    bl2 = np.asarray(inputs["ln2_b"], np.float32)

    s = np.float32(D ** -0.5)
    x_flat = x.reshape(BT, D)
    xt = np.ascontiguousarray(
        x_flat.reshape(NBLK, 512, NDC, 128).transpose(0, 3, 2, 1)
    ).astype(ml_dtypes.bfloat16)

    # additive causal mask bias for the diagonal 128x128 window:
    # key i (partition), query col j: 0 if j >= i else MASKB
    jj = np.arange(128)[None, :]
    ii = np.arange(128)[:, None]
    maskb = np.where(jj >= ii, 0.0, MASKB).astype(np.float32)

    W1g = W1 * g2[:, None]
    w1fix = np.stack([W1g.sum(0), bl2 @ W1 + b1]).astype(np.float32)

    common = {
        "xt": xt,
        "wproj": np.ascontiguousarray(Wproj.reshape(NDC, 128, D).transpose(1, 0, 2)).astype(ml_dtypes.bfloat16),
        "w1": np.ascontiguousarray(W1g.reshape(NDC, 128, NHT, 128).transpose(2, 1, 0, 3)).astype(ml_dtypes.bfloat16),
        "w1fix": w1fix,
        "w2": np.ascontiguousarray(W2.reshape(NHT, 128, NDC, 128).transpose(2, 1, 0, 3)).astype(ml_dtypes.bfloat16),
        "bproj_pp": np.ascontiguousarray(bproj.reshape(8, 128).T),
        "b2p_pp": np.ascontiguousarray((bl2 + b2).reshape(8, 128).T),
        "b1ln_pp": np.ascontiguousarray(bl1.reshape(8, 128).T),
        "g1diag": np.ascontiguousarray(
            (np.eye(128)[None] * g1.reshape(NDC, 1, 128)).transpose(1, 0, 2)).astype(np.float32),
        "g1row": g1.reshape(1, D),
        "g2diag": np.ascontiguousarray(
            (np.eye(128)[None] * g2.reshape(NDC, 1, 128)).transpose(1, 0, 2)).astype(ml_dtypes.bfloat16),
        "g2row": g2.reshape(1, D),
        "maskb": maskb.astype(ml_dtypes.bfloat16),
        "identb": np.eye(128, dtype=np.float32).astype(ml_dtypes.bfloat16),
        "ident": np.eye(128, dtype=np.float32),
        "ones_d": np.full((128, 1), 1.0 / D, np.float32),
        "ones_db": np.full((128, 1), 1.0 / D, ml_dtypes.bfloat16),
        "ones_1": np.ones((1, 128), np.float32),
        "ones_1b": np.ones((1, 128), ml_dtypes.bfloat16),
    }

    in_maps = []
    for c in range(NC_):
        h0 = NH_LOC * c
        Wq_cat = np.concatenate([Wq[h0 + i] for i in range(NH_LOC)], 1)  # [D,128]
        Wk_cat = np.concatenate([Wk[h0 + i] for i in range(NH_LOC)], 1)
        Wv_cat = np.concatenate([Wv[h0 + i] for i in range(NH_LOC)], 1)
        Wq_eff = g1[:, None] * Wq_cat * s
        Wk_eff = g1[:, None] * Wk_cat
        Wv_eff = g1[:, None] * Wv_cat
        wqkv = np.ascontiguousarray(np.concatenate([Wq_eff, Wk_eff, Wv_eff], 1)).astype(ml_dtypes.bfloat16)
        wsums = np.concatenate([Wq_eff.sum(0), Wk_eff.sum(0), Wv_eff.sum(0)])
        wbias = np.concatenate([bl1 @ (Wq_cat * s), bl1 @ Wk_cat, bl1 @ Wv_cat])
        m = dict(common)
        xl = np.concatenate([x[0, 256 * c:256 * (c + 1)],
                             x[1, 256 * c:256 * (c + 1)]], axis=0)  # [512, D]
        m["xt_loc"] = np.ascontiguousarray(xl.reshape(512, NDC, 128).transpose(2, 1, 0))
        m["wqkv"] = wqkv
        m["wsb2"] = np.ascontiguousarray(
            np.stack([wsums, wbias]).astype(np.float32))
        m["ws_row"] = np.ascontiguousarray(wsums.reshape(1, 384).astype(np.float32))
        m["wb_row"] = np.ascontiguousarray(wbias.reshape(1, 384).astype(np.float32))
        in_maps.append(m)
    return in_maps


def run(inputs, trace=False, trace_kwargs=None):
    nc = _get_nc()
    in_maps = make_in_maps(inputs)
    res = run_bass_kernel_spmd(nc, in_maps, core_ids=list(range(NC_)),
                               trace=trace, **(trace_kwargs or {}))
    out = np.empty((B, T, D), np.float32)
    for c in range(NC_):
        o = res.results[c]["outT"]
        out[0, 256 * c:256 * (c + 1)] = o[:, 0:256].T
        out[1, 256 * c:256 * (c + 1)] = o[:, 256:512].T
    return out, res


def kernel(**inputs) -> np.ndarray:
    out, _ = run(inputs, trace=False)
    return out


# revision 40
# speedup vs baseline: 1.1012x; 1.1012x over previous
"""Trainium2 Bass/Tile kernel for a dense transformer block (B=2, T=2048, D=1024, H=16).

Sharding across 8 NeuronCores (head-parallel attention + token-parallel FFN):
  - LN1 statistics: each core computes stats for its own 512 tokens, then a
    tiny AllGather ([3,512] -> [24,512]) replicates (-mu, sd, 1/sd) rows.
  - LN1 *apply* is folded into the PE: raw QKV psums get one extra accumulated
    "fix" matmul (wsums^T @ [-mu; sd]); Q is then scaled by the broadcast A row
    (one DVE op); K stays unscaled and its per-key scale A_k is applied as the
    per-partition `scale` of the exp activation; V stays unscaled and A_k is
    folded into the per-key 1/c normalizer.
  - Attention is head-sharded (2 heads/core, both batches). Query-axis softmax:
      attn^T = (V*A/c)^T @ exp(A_k * (S^T + mask_bias)),  c[k] = row-sum
    with the causal mask applied as an additive -30 bias via one PE matmul,
    so each (key-tile, head) needs exactly ONE exp activation (+accum for c).
  - One 2MB/core AllToAll per batch reshards attention output to token-split.
  - Projection + residual, LN2 and the FFN run token-sharded; the LN2 apply,
    b1/b2 biases and the LN2 gamma are all folded into W1/W2 fix matmuls on
    the PE (relu(A2*x) = A2*relu(x) since A2>0), with the A2 column scale
    applied once at the output stage.
Everything is feature-major ("transposed") so every matmul has a natural lhsT.
"""

import numpy as np
import ml_dtypes

import concourse.bass as bass
import concourse.bacc as bacc
import concourse.mybir as mybir
import concourse.tile as tile
from concourse.bass_utils import run_bass_kernel_spmd

F32 = mybir.dt.float32
BF16 = mybir.dt.bfloat16
AF = mybir.ActivationFunctionType
ALU = mybir.AluOpType

B, T, D, H = 2, 2048, 1024, 16
HS = D // H          # 64
DFF = 4 * D          # 4096
EPS = 1e-5
NC_ = 8              # cores
BT = B * T           # 4096 flat tokens
TOK = BT // NC_      # 512 tokens per core
NBLK = BT // 512     # 8 token blocks
NDC = D // 128       # 8 d-chunks
NH_LOC = H // NC_    # 2 heads per core
NKT = T // 128       # 16 key tiles per batch
NHT = DFF // 128     # 32 hidden tiles
MASKB = -30.0        # additive pre-exp mask bias


def _build_nc():
    nc = bacc.Bacc(num_devices=NC_)

    xt_d = nc.dram_tensor("xt", [NBLK, 128, NDC, 512], BF16, kind="ExternalInput")
    xtloc_d = nc.dram_tensor("xt_loc", [128, NDC, 512], F32, kind="ExternalInput")
    wqkv_d = nc.dram_tensor("wqkv", [D, 384], BF16, kind="ExternalInput")
    wsb2_d = nc.dram_tensor("wsb2", [2, 384], F32, kind="ExternalInput")
    wsr_d = nc.dram_tensor("ws_row", [1, 384], F32, kind="ExternalInput")
    wbr_d = nc.dram_tensor("wb_row", [1, 384], F32, kind="ExternalInput")
    wproj_d = nc.dram_tensor("wproj", [128, NDC, D], BF16, kind="ExternalInput")
    w1_d = nc.dram_tensor("w1", [NHT, 128, NDC, 128], BF16, kind="ExternalInput")
    w1fix_d = nc.dram_tensor("w1fix", [2, DFF], F32, kind="ExternalInput")
    w2_d = nc.dram_tensor("w2", [NDC, 128, NHT, 128], BF16, kind="ExternalInput")
    bproj_d = nc.dram_tensor("bproj_pp", [128, 8], F32, kind="ExternalInput")
    b2p_d = nc.dram_tensor("b2p_pp", [128, 8], F32, kind="ExternalInput")
    b1ln_d = nc.dram_tensor("b1ln_pp", [128, 8], F32, kind="ExternalInput")
    g1diag_d = nc.dram_tensor("g1diag", [128, NDC, 128], F32, kind="ExternalInput")
    g1row_d = nc.dram_tensor("g1row", [1, D], F32, kind="ExternalInput")
    g2diag_d = nc.dram_tensor("g2diag", [128, NDC, 128], BF16, kind="ExternalInput")
    g2row_d = nc.dram_tensor("g2row", [1, D], F32, kind="ExternalInput")
    maskb_d = nc.dram_tensor("maskb", [128, 128], BF16, kind="ExternalInput")
    identb_d = nc.dram_tensor("identb", [128, 128], BF16, kind="ExternalInput")
    ident_d = nc.dram_tensor("ident", [128, 128], F32, kind="ExternalInput")
    onesd_d = nc.dram_tensor("ones_d", [128, 1], F32, kind="ExternalInput")    # 1/D
    onesdb_d = nc.dram_tensor("ones_db", [128, 1], BF16, kind="ExternalInput")  # 1/D bf16
    ones1_d = nc.dram_tensor("ones_1", [1, 128], F32, kind="ExternalInput")
    ones1b_d = nc.dram_tensor("ones_1b", [1, 128], BF16, kind="ExternalInput")
    out_d = nc.dram_tensor("outT", [D, TOK], F32, kind="ExternalOutput")

    with tile.TileContext(nc) as tc:
        with tc.tile_pool(name="const", bufs=1) as cst, \
             tc.tile_pool(name="dram", bufs=1, space="DRAM") as dpool:
            def cload(shape, dram_ap, dtype=F32):
                t = cst.tile(shape, dtype, name=f"c{len(nc.m.functions[0].allocations)}")
                nc.gpsimd.dma_start(t[:], dram_ap)
                return t

            # dummy collective FIRST: absorbs the ~30us first-collective
            # firmware barrier while real work proceeds (result unused)
            dum_in = dpool.tile([1, 8], BF16, name="dum_i")
            dum_out = dpool.tile([8, 8], BF16, name="dum_o")

            # ---- HAM warmup: keep PE busy while constants stream in ----
            with tc.tile_pool(name="warm", bufs=1) as wp, \
                 tc.tile_pool(name="warm_ps", bufs=1, space="PSUM") as wps:
                wt_ = wp.tile([128, 512], BF16)
                nc.vector.memset(wt_[:], 0.001)
                nc.sync.dma_start(dum_in[:], wt_[0:1, 0:8])
                nc.gpsimd.collective_compute(
                    "AllGather", ALU.bypass,
                    replica_groups=[list(range(NC_))],
                    ins=[dum_in.opt()], outs=[dum_out.opt()])
                wpt = wps.tile([128, 512], F32)
                for _ in range(16):
                    nc.tensor.matmul(wpt[:], wt_[:, 0:128], wt_[:], start=True, stop=True)

            wqkv_sb = cload([128, NDC, 384], wqkv_d[:, :].rearrange("(a p) m -> p a m", p=128), BF16)
            maskb_sb = cload([128, 128], maskb_d[:, :], BF16)
            identb_sb = cload([128, 128], identb_d[:, :], BF16)
            ident_sb = cload([128, 128], ident_d[:, :])
            onesd_sb = cload([128, 1], onesd_d[:, :])
            onesdb_sb = cload([128, 1], onesdb_d[:, :], BF16)
            ones1_sb = cload([1, 128], ones1_d[:, :])
            wsb2_sb = cload([2, 384], wsb2_d[:, :])
            wsr_sb = cload([1, 384], wsr_d[:, :])
            wbr_sb = cload([1, 384], wbr_d[:, :])
            ones1b_sb = cload([1, 128], ones1b_d[:, :], BF16)
            g1diag_sb = cload([128, NDC, 128], g1diag_d[:, :, :])
            g1row_sb = cload([1, D], g1row_d[:, :])
            g2diag_sb = cload([128, NDC, 128], g2diag_d[:, :, :], BF16)
            g2row_sb = cload([1, D], g2row_d[:, :])
            bproj_sb = cload([128, 8], bproj_d[:, :])
            b2p_sb = cload([128, 8], b2p_d[:, :])
            b1ln_sb = cload([128, 8], b1ln_d[:, :])
            w1fix_sb = cload([2, DFF], w1fix_d[:, :])

            xw = cst.tile([128, NDC, 512], F32)      # x_loc -> xn_loc -> x1_loc
            x1b = cst.tile([128, NDC, 512], BF16)    # bf16 copy of x1 for FFN
            stats2 = cst.tile([2, 512], F32)         # LN2 rows (-mu2; sd2)
            a2bc = cst.tile([128, 512], F32)         # broadcast A2 row
            Gr_sb = cst.tile([2, NC_, 512], F32)     # gathered (-mu, sd) rows per rank
            Ar_sb = cst.tile([1, NC_, 512], F32)     # gathered 1/sd rows per rank
            GT_sb = cst.tile([128, 4, 24], F32)      # transposed stats (A_k columns)
            hT = cst.tile([128, NHT, 512], BF16)     # FFN hidden
            attn_loc = cst.tile([128, NDC, 512], BF16)

            nc.sync.dma_start(xw[:], xtloc_d[:, :, :])

            # ===== local LN1 stats for my 512 tokens + AllGather =====
            with (
                tc.tile_pool(name="st_sm", bufs=1) as stp,
                tc.tile_pool(name="st_ps", bufs=1, space="PSUM") as stps,
            ):
                s1p = stps.tile([1, 512], F32)
                s2p = stps.tile([1, 512], F32)
                for a in range(NDC):
                    sq = stp.tile([128, 512], BF16, tag="sq")
                    if a % 2 == 0:
                        nc.scalar.square(sq[:], xw[:, a, :])
                    else:
                        nc.vector.tensor_tensor(sq[:], xw[:, a, :], xw[:, a, :], ALU.mult)
                    nc.tensor.matmul(s1p[:], onesd_sb[:], xw[:, a, :],
                                     start=(a == 0), stop=(a == NDC - 1))
                    nc.tensor.matmul(s2p[:], onesdb_sb[:], sq[:],
                                     start=(a == 0), stop=(a == NDC - 1))
                nmu_l = stp.tile([1, 512], F32)   # -mu
                sd_l = stp.tile([1, 512], F32)    # sd
                arow_l = stp.tile([1, 512], F32)  # 1/sd
                msq = stp.tile([1, 512], F32)
                nc.scalar.square(msq[:], s1p[:])
                veps = stp.tile([1, 512], F32)
                nc.vector.scalar_tensor_tensor(veps[:], s2p[:], EPS, msq[:],
                                               ALU.add, ALU.subtract)
                nc.scalar.sqrt(sd_l[:], veps[:])
                nc.vector.reciprocal(arow_l[:], sd_l[:])
                nc.vector.tensor_scalar_mul(nmu_l[:], s1p[:], -1.0)

                ag_in = dpool.tile([3, 512], F32, name="ag_in")
                ag_out = dpool.tile([24, 512], F32, name="ag_out")
                nc.sync.dma_start(ag_in[0:1], nmu_l[:])
                nc.sync.dma_start(ag_in[1:2], sd_l[:])
                nc.sync.dma_start(ag_in[2:3], arow_l[:])
                nc.gpsimd.collective_compute(
                    "AllGather", ALU.bypass,
                    replica_groups=[list(range(NC_))],
                    ins=[ag_in.opt()], outs=[ag_out.opt()])
                ag_r = ag_out[:, :].rearrange("(r s) n -> s r n", s=3)
                nc.sync.dma_start(Gr_sb[:], ag_r[0:2])
                nc.sync.dma_start(Ar_sb[:], ag_r[2:3])
                G24 = stp.tile([24, 512], F32)
                nc.sync.dma_start(G24[:], ag_out[:, :])
                for c4 in range(4):
                    gtp = stps.tile([128, 24], F32, tag="gtp", bufs=2)
                    nc.tensor.transpose(gtp[:], G24[:, 128 * c4:128 * (c4 + 1)],
                                        ident_sb[0:24, 0:24])
                    nc.vector.tensor_copy(GT_sb[:, c4, :], gtp[:])

                # ---- LN1 apply on local slice: xn = Abc*(g1*x + g1*(-mu)) + b1ln ----
                abl = stps.tile([128, 512], F32)
                nc.tensor.matmul(abl[:], ones1_sb[:], arow_l[:], start=True, stop=True)
                abl_sb = stp.tile([128, 512], F32)
                nc.scalar.copy(abl_sb[:], abl[:])
                for a in range(NDC):
                    pa = stps.tile([128, 512], F32, tag="pa", bufs=2)
                    nc.tensor.matmul(pa[:], g1diag_sb[:, a, :], xw[:, a, :],
                                     start=True, stop=False)
                    nc.tensor.matmul(pa[:], g1row_sb[:, 128 * a:128 * (a + 1)],
                                     nmu_l[:], start=False, stop=True)
                    t1 = stp.tile([128, 512], F32, tag="t1", bufs=2)
                    nc.vector.tensor_tensor(t1[:], pa[:], abl_sb[:], ALU.mult)
                    nc.scalar.activation(xw[:, a, :], t1[:], AF.Identity,
                                         bias=b1ln_sb[:, a:a + 1])

            # ===== big attention scope =====
            with tc.tile_pool(name="attn_scope", bufs=1) as big:
                Q2 = big.tile([128, BT], BF16)
                K2 = big.tile([128, BT], BF16)
                VK = big.tile([128, BT], BF16)   # k-major V, heads side by side
                attnT = big.tile([128, BT], BF16)

                # ===== Phase A: raw QKV + PE-folded LN fixups =====
                with (
                    tc.tile_pool(name="sweep", bufs=2) as swp,
                    tc.tile_pool(name="sweep_ps", bufs=2, space="PSUM") as swps,
                    tc.tile_pool(name="fix_ps", bufs=1, space="PSUM") as fxps,
                ):
                    for tb in range(NBLK):
                        xblk = swp.tile([128, NDC, 512], BF16, tag="xblk")
                        nc.sync.dma_start(xblk[:], xt_d[tb])
                        # rank/col-offset of this block's two 256-token halves
                        halves = [(2 * (tb % 4) + h, 256 * (tb // 4)) for h in range(2)]
                        qkps = []
                        for j in range(3):
                            ps = swps.tile([128, 512], F32, tag="qkvps", bufs=4)
                            for a in range(NDC):
                                nc.tensor.matmul(ps[:], wqkv_sb[:, a, 128 * j:128 * (j + 1)],
                                                 xblk[:, a, :],
                                                 start=(a == 0), stop=(a == NDC - 1))
                            for h, (r, c0) in enumerate(halves):
                                nc.tensor.matmul(
                                    ps[:, 256 * h:256 * (h + 1)],
                                    wsb2_sb[:, 128 * j:128 * (j + 1)],
                                    Gr_sb[:, r, c0:c0 + 256],
                                    start=False, stop=(h == 1), skip_group_check=True)
                            qkps.append(ps)
                        bap = fxps.tile([128, 512], F32, tag="bap")
                        for h, (r, c0) in enumerate(halves):
                            nc.tensor.matmul(bap[:, 256 * h:256 * (h + 1)], ones1_sb[:],
                                             Ar_sb[:, r, c0:c0 + 256],
                                             start=(h == 0), stop=(h == 1),
                                             skip_group_check=True)
                        ba_sb = swp.tile([128, 512], F32, tag="ba_sb")
                        nc.scalar.copy(ba_sb[:], bap[:])
                        vsb = swp.tile([128, 512], F32, tag="vsb")
                        nc.scalar.copy(vsb[:], qkps[2][:])
                        ts_ = slice(512 * tb, 512 * (tb + 1))
                        nc.scalar.copy(K2[:, ts_], qkps[1][:])
                        nc.vector.tensor_tensor(Q2[:, ts_], qkps[0][:], ba_sb[:], ALU.mult)
                        for q in range(4):
                            vtp = fxps.tile([128, 128], F32, tag="vtp", bufs=2)
                            nc.tensor.transpose(vtp[:], vsb[:, 128 * q:128 * (q + 1)],
                                                ident_sb[:])
                            nc.vector.tensor_copy(
                                VK[:, 512 * tb + 128 * q:512 * tb + 128 * (q + 1)],
                                vtp[:])

                # ===== Phase B: attention per (batch, key-tile), A2A per batch =====
                with (
                    tc.tile_pool(name="se_pool", bufs=2) as sep,
                    tc.tile_pool(name="attn_sm", bufs=2) as asm,
                    tc.tile_pool(name="st_ps2", bufs=1, space="PSUM") as sps2,
                    tc.tile_pool(name="ap_ps", bufs=1, space="PSUM") as apps,
                ):
                    for b in range(B):
                        ap_tiles = [apps.tile([128, 512], F32, name=f"app{b}_{qb}",
                                              tag=f"ap{qb}", bufs=1)
                                    for qb in range(4)]
                        se_tiles, vc_tiles, st_tiles, ck_tiles = {}, {}, {}, {}

                        def ascale(kt):
                            return GT_sb[:, 2 * b + (kt % 2), 3 * (kt // 2) + 2:
                                         3 * (kt // 2) + 3]

                        def st_mms(kt, hh, c):
                            # chunk c covers region cols [1024c, min(1024(c+1), W))
                            qb0 = kt // 4
                            co = 128 * (kt % 4)
                            W = 2048 - 512 * qb0
                            q0 = b * T + 512 * qb0
                            stp_ = sps2.tile([128, 1024], F32, tag=f"st{hh}", bufs=1)
                            st_tiles[(kt, hh, c)] = stp_
                            hsl = slice(64 * hh, 64 * hh + 64)
                            lo = max(co, 1024 * c)
                            hi = min(W, 1024 * (c + 1))
                            g = lo
                            while g < hi:
                                g2 = min(hi, (g // 512 + 1) * 512)
                                nc.tensor.matmul(
                                    stp_[:, g - 1024 * c:g2 - 1024 * c],
                                    K2[hsl, b * T + 128 * kt:b * T + 128 * (kt + 1)],
                                    Q2[hsl, q0 + g:q0 + g2],
                                    start=True, stop=False,
                                    skip_group_check=True)
                                g = g2
                            if c == 0:
                                nc.tensor.matmul(
                                    stp_[:, co:co + 128], identb_sb[:],
                                    maskb_sb[:, :],
                                    start=False, stop=True, skip_group_check=True)

                        def exp_chunk(kt, hh, c, nchunk):
                            qb0 = kt // 4
                            co = 128 * (kt % 4)
                            W = 2048 - 512 * qb0
                            stp_ = st_tiles.pop((kt, hh, c))
                            lo = max(co, 1024 * c)
                            hi = min(W, 1024 * (c + 1))
                            if c == 0:
                                se = sep.tile([128, 2048], BF16,
                                              name=f"se_{b}_{kt}_{hh}",
                                              tag=f"se{hh}", bufs=3)
                                se_tiles[(kt, hh)] = se
                            se = se_tiles[(kt, hh)]
                            nc.scalar.activation(
                                se[:, lo:hi], stp_[:, lo - 1024 * c:hi - 1024 * c],
                                AF.Exp, scale=ascale(kt))
                            if c == nchunk - 1:
                                co = 128 * (kt % 4)
                                W = 2048 - 512 * (kt // 4)
                                ck = asm.tile([128, 1], F32, tag=f"ck{hh}", bufs=2)
                                nc.vector.tensor_reduce(ck[:], se[:, co:W],
                                                        mybir.AxisListType.X, ALU.add)
                                rk = asm.tile([128, 1], F32, tag=f"rk{hh}", bufs=2)
                                nc.vector.reciprocal(rk[:], ck[:])
                                ark = asm.tile([128, 1], F32, tag=f"ark{hh}", bufs=2)
                                nc.vector.tensor_tensor(ark[:], rk[:], ascale(kt),
                                                        ALU.mult)
                                vc = asm.tile([128, 64], BF16, tag=f"vc{hh}", bufs=2)
                                ktf = b * NKT + kt
                                nc.vector.tensor_scalar_mul(
                                    vc[:],
                                    VK[:, 128 * ktf + 64 * hh:128 * ktf + 64 * hh + 64],
                                    ark[:])
                                vc_tiles[(kt, hh)] = vc

                        def emit_at(kt):
                            qb0 = kt // 4
                            co = 128 * (kt % 4)
                            for qb in range(qb0, 4):
                                off = 512 * (qb - qb0)
                                n0 = co if qb == qb0 else 0
                                last = (kt == 4 * (qb + 1) - 1)
                                for hh in range(NH_LOC):
                                    nc.tensor.matmul(
                                        ap_tiles[qb][64 * hh:64 * hh + 64, n0:512],
                                        vc_tiles[(kt, hh)][:],
                                        se_tiles[(kt, hh)][:, off + n0:off + 512],
                                        start=(kt == 0), stop=last,
                                        tile_position=(0, 64 * hh),
                                        skip_group_check=True)
                                if last:
                                    nc.vector.tensor_copy(
                                        attnT[:, b * T + 512 * qb:b * T + 512 * (qb + 1)],
                                        ap_tiles[qb][:])

                        for kt in range(NKT + 1):
                            if kt < NKT:
                                W = 2048 - 512 * (kt // 4)
                                nchunk = (W + 1023) // 1024
                                for c in range(nchunk):
                                    st_mms(kt, 0, c)
                                    st_mms(kt, 1, c)
                                    exp_chunk(kt, 0, c, nchunk)
                                    exp_chunk(kt, 1, c, nchunk)
                            if kt >= 1:
                                emit_at(kt - 1)

                        a2a_in = dpool.tile([8, 128, 256], BF16, name=f"a2ai{b}", tag=f"a2ai{b}")
                        a2a_out = dpool.tile([8, 128, 256], BF16, name=f"a2ao{b}", tag=f"a2ao{b}")
                        for j in range(8):
                            nc.sync.dma_start(a2a_in[j],
                                              attnT[:, b * T + 256 * j:b * T + 256 * (j + 1)])
                        nc.gpsimd.collective_compute(
                            "AllToAll", ALU.bypass,
                            replica_groups=[list(range(NC_))],
                            ins=[a2a_in.opt()], outs=[a2a_out.opt()])
                        for s in range(8):
                            nc.sync.dma_start(attn_loc[:, s, 256 * b:256 * (b + 1)], a2a_out[s])

            # ===== Phase D: projection + residual + LN2 stats (PE-folded apply) =====
            with (
                tc.tile_pool(name="proj_sm", bufs=2) as pjm,
                tc.tile_pool(name="proj_ps", bufs=2, space="PSUM") as pjps,
            ):
                for dt in range(NDC):
                    wpt = pjm.tile([128, NDC, 128], BF16, tag="wp", bufs=3)
                    nc.sync.dma_start(wpt[:], wproj_d[:, :, 128 * dt:128 * (dt + 1)])
                    pp = pjps.tile([128, 512], F32, tag="pp")
                    for a in range(NDC):
                        nc.tensor.matmul(pp[:], wpt[:, a, :],
                                         attn_loc[:, a, :],
                                         start=(a == 0), stop=(a == NDC - 1))
                    nc.vector.scalar_tensor_tensor(
                        xw[:, dt, :], pp[:], bproj_sb[:, dt:dt + 1], xw[:, dt, :],
                        ALU.add, ALU.add)
                    nc.scalar.copy(x1b[:, dt, :], xw[:, dt, :])
                # LN2 stats on x1
                s1p = pjps.tile([1, 512], F32, tag="s1b", bufs=1)
                s2p = pjps.tile([1, 512], F32, tag="s2b", bufs=1)
                for a in range(NDC):
                    sq = pjm.tile([128, 512], BF16, tag="sq2")
                    nc.vector.tensor_tensor(sq[:], x1b[:, a, :], x1b[:, a, :], ALU.mult)
                    nc.tensor.matmul(s1p[:], onesd_sb[:], xw[:, a, :],
                                     start=(a == 0), stop=(a == NDC - 1))
                    nc.tensor.matmul(s2p[:], onesdb_sb[:], sq[:],
                                     start=(a == 0), stop=(a == NDC - 1))
                msq = pjm.tile([1, 512], F32, tag="msq2", bufs=1)
                nc.scalar.square(msq[:], s1p[:])
                veps = pjm.tile([1, 512], F32, tag="veps2", bufs=1)
                nc.vector.scalar_tensor_tensor(veps[:], s2p[:], EPS, msq[:],
                                               ALU.add, ALU.subtract)
                sd2_l = pjm.tile([1, 512], F32, tag="sd2l", bufs=1)
                nc.scalar.sqrt(sd2_l[:], veps[:])
                a2row = pjm.tile([1, 512], F32, tag="a2row", bufs=1)
                nc.vector.reciprocal(a2row[:], sd2_l[:])
                nc.vector.tensor_scalar_mul(stats2[0:1, :], s1p[:], -1.0)
                nc.sync.dma_start(stats2[1:2, :], sd2_l[:])
                a2p = pjps.tile([128, 512], F32, tag="a2p", bufs=1)
                nc.tensor.matmul(a2p[:], ones1_sb[:], a2row[:], start=True, stop=True)
                nc.scalar.copy(a2bc[:], a2p[:])

            # ===== Phase E: FFN (token-sharded, streamed weights) =====
            with (
                tc.tile_pool(name="ffn_w1", bufs=3) as w1p,
                tc.tile_pool(name="ffn_w2", bufs=2) as w2p,
                tc.tile_pool(name="ffn_sm", bufs=2) as fsm,
                tc.tile_pool(name="ffn_ps", bufs=2, space="PSUM") as fps,
            ):
                for ht in range(NHT):
                    w1t = w1p.tile([128, NDC, 128], BF16, tag="w1", bufs=4)
                    nc.sync.dma_start(w1t[:], w1_d[ht])
                    hp = fps.tile([128, 512], F32, tag="hp")
                    for a in range(NDC):
                        nc.tensor.matmul(hp[:], w1t[:, a, :], x1b[:, a, :],
                                         start=(a == 0), stop=False)
                    nc.tensor.matmul(hp[:], w1fix_sb[:, 128 * ht:128 * (ht + 1)],
                                     stats2[:, :], start=False, stop=True)
                    nc.scalar.activation(hT[:, ht, :], hp[:], AF.Relu)
                for dt in range(NDC):
                    w2t = w2p.tile([128, NHT, 128], BF16, tag="w2")
                    nc.sync.dma_start(w2t[:], w2_d[dt])
                    fp_ = fps.tile([128, 512], F32, tag="fp")
                    for a2_ in range(NHT):
                        nc.tensor.matmul(fp_[:], w2t[:, a2_, :], hT[:, a2_, :],
                                         start=(a2_ == 0), stop=False)
                    nc.tensor.matmul(fp_[:], g2diag_sb[:, dt, :], x1b[:, dt, :],
                                     start=False, stop=False)
                    nc.tensor.matmul(fp_[:], g2row_sb[:, 128 * dt:128 * (dt + 1)],
                                     stats2[0:1, :], start=False, stop=True)
                    t1 = fsm.tile([128, 512], F32, tag="t1f")
                    nc.vector.tensor_tensor(t1[:], fp_[:], a2bc[:], ALU.mult)
                    ot = fsm.tile([128, 512], F32, tag="ot")
                    nc.scalar.activation(ot[:], t1[:], AF.Identity,
                                         bias=b2p_sb[:, dt:dt + 1])
                    nc.sync.dma_start(
                        out_d[:, :].rearrange("(a p) n -> p a n", p=128)[:, dt, :], ot[:])
    nc.compile()
    return nc


_NC_CACHE = None


def _get_nc():
    global _NC_CACHE
    if _NC_CACHE is None:
        _NC_CACHE = _build_nc()
    return _NC_CACHE


def make_in_maps(inputs):
    x = np.asarray(inputs["x"], np.float32)
    Wq = np.asarray(inputs["Wq"], np.float32)
    Wk = np.asarray(inputs["Wk"], np.float32)
    Wv = np.asarray(inputs["Wv"], np.float32)
    Wproj = np.ascontiguousarray(np.asarray(inputs["Wproj"], np.float32))
    bproj = np.asarray(inputs["bproj"], np.float32)
    W1 = np.ascontiguousarray(np.asarray(inputs["W1"], np.float32))
    b1 = np.asarray(inputs["b1"], np.float32)
    W2 = np.ascontiguousarray(np.asarray(inputs["W2"], np.float32))
    b2 = np.asarray(inputs["b2"], np.float32)
    g1 = np.asarray(inputs["ln1_g"], np.float32)
    bl1 = np.asarray(inputs["ln1_b"], np.float32)
    g2 = np.asarray(inputs["ln2_g"], np.float32)
    bl2 = np.asarray(inputs["ln2_b"], np.float32)

    s = np.float32(D ** -0.5)
    x_flat = x.reshape(BT, D)
    xt = np.ascontiguousarray(
        x_flat.reshape(NBLK, 512, NDC, 128).transpose(0, 3, 2, 1)
    ).astype(ml_dtypes.bfloat16)

    # additive causal mask bias for the diagonal 128x128 window:
    # key i (partition), query col j: 0 if j >= i else MASKB
    jj = np.arange(128)[None, :]
    ii = np.arange(128)[:, None]
    maskb = np.where(jj >= ii, 0.0, MASKB).astype(np.float32)

    W1g = W1 * g2[:, None]
    w1fix = np.stack([W1g.sum(0), bl2 @ W1 + b1]).astype(np.float32)

    common = {
        "xt": xt,
        "wproj": np.ascontiguousarray(Wproj.reshape(NDC, 128, D).transpose(1, 0, 2)).astype(ml_dtypes.bfloat16),
        "w1": np.ascontiguousarray(W1g.reshape(NDC, 128, NHT, 128).transpose(2, 1, 0, 3)).astype(ml_dtypes.bfloat16),
        "w1fix": w1fix,
        "w2": np.ascontiguousarray(W2.reshape(NHT, 128, NDC, 128).transpose(2, 1, 0, 3)).astype(ml_dtypes.bfloat16),
        "bproj_pp": np.ascontiguousarray(bproj.reshape(8, 128).T),
        "b2p_pp": np.ascontiguousarray((bl2 + b2).reshape(8, 128).T),
        "b1ln_pp": np.ascontiguousarray(bl1.reshape(8, 128).T),
        "g1diag": np.ascontiguousarray(
            (np.eye(128)[None] * g1.reshape(NDC, 1, 128)).transpose(1, 0, 2)).astype(np.float32),
        "g1row": g1.reshape(1, D),
        "g2diag": np.ascontiguousarray(
            (np.eye(128)[None] * g2.reshape(NDC, 1, 128)).transpose(1, 0, 2)).astype(ml_dtypes.bfloat16),
        "g2row": g2.reshape(1, D),
        "maskb": maskb.astype(ml_dtypes.bfloat16),
        "identb": np.eye(128, dtype=np.float32).astype(ml_dtypes.bfloat16),
        "ident": np.eye(128, dtype=np.float32),
        "ones_d": np.full((128, 1), 1.0 / D, np.float32),
        "ones_db": np.full((128, 1), 1.0 / D, ml_dtypes.bfloat16),
        "ones_1": np.ones((1, 128), np.float32),
        "ones_1b": np.ones((1, 128), ml_dtypes.bfloat16),
    }

    in_maps = []
    for c in range(NC_):
        h0 = NH_LOC * c
        Wq_cat = np.concatenate([Wq[h0 + i] for i in range(NH_LOC)], 1)  # [D,128]
        Wk_cat = np.concatenate([Wk[h0 + i] for i in range(NH_LOC)], 1)
        Wv_cat = np.concatenate([Wv[h0 + i] for i in range(NH_LOC)], 1)
        Wq_eff = g1[:, None] * Wq_cat * s
        Wk_eff = g1[:, None] * Wk_cat
        Wv_eff = g1[:, None] * Wv_cat
        wqkv = np.ascontiguousarray(np.concatenate([Wq_eff, Wk_eff, Wv_eff], 1)).astype(ml_dtypes.bfloat16)
        wsums = np.concatenate([Wq_eff.sum(0), Wk_eff.sum(0), Wv_eff.sum(0)])
        wbias = np.concatenate([bl1 @ (Wq_cat * s), bl1 @ Wk_cat, bl1 @ Wv_cat])
        m = dict(common)
        xl = np.concatenate([x[0, 256 * c:256 * (c + 1)],
                             x[1, 256 * c:256 * (c + 1)]], axis=0)  # [512, D]
        m["xt_loc"] = np.ascontiguousarray(xl.reshape(512, NDC, 128).transpose(2, 1, 0))
        m["wqkv"] = wqkv
        m["wsb2"] = np.ascontiguousarray(
            np.stack([wsums, wbias]).astype(np.float32))
        m["ws_row"] = np.ascontiguousarray(wsums.reshape(1, 384).astype(np.float32))
        m["wb_row"] = np.ascontiguousarray(wbias.reshape(1, 384).astype(np.float32))
        in_maps.append(m)
    return in_maps


def run(inputs, trace=False, trace_kwargs=None):
    nc = _get_nc()
    in_maps = make_in_maps(inputs)
    res = run_bass_kernel_spmd(nc, in_maps, core_ids=list(range(NC_)),
                               trace=trace, **(trace_kwargs or {}))
    out = np.empty((B, T, D), np.float32)
    for c in range(NC_):
        o = res.results[c]["outT"]
        out[0, 256 * c:256 * (c + 1)] = o[:, 0:256].T
        out[1, 256 * c:256 * (c + 1)] = o[:, 256:512].T
    return out, res


def kernel(**inputs) -> np.ndarray:
    out, _ = run(inputs, trace=False)
    return out


# revision 42
# speedup vs baseline: 1.1375x; 1.0331x over previous
"""Trainium2 Bass/Tile kernel for a dense transformer block (B=2, T=2048, D=1024, H=16).

Sharding across 8 NeuronCores (head-parallel attention + token-parallel FFN):
  - LN1 statistics: each core computes stats for its own 512 tokens, then a
    tiny AllGather ([3,512] -> [24,512]) replicates (-mu, sd, 1/sd) rows.
  - LN1 *apply* is folded into the PE: raw QKV psums get one extra accumulated
    "fix" matmul (wsums^T @ [-mu; sd]); Q is then scaled by the broadcast A row
    (one DVE op); K stays unscaled and its per-key scale A_k is applied as the
    per-partition `scale` of the exp activation; V stays unscaled and A_k is
    folded into the per-key 1/c normalizer.
  - Attention is head-sharded (2 heads/core, both batches). Query-axis softmax:
      attn^T = (V*A/c)^T @ exp(A_k * (S^T + mask_bias)),  c[k] = row-sum
    with the causal mask applied as an additive -30 bias via one PE matmul,
    so each (key-tile, head) needs exactly ONE exp activation (+accum for c).
  - One 2MB/core AllToAll per batch reshards attention output to token-split.
  - Projection + residual, LN2 and the FFN run token-sharded; the LN2 apply,
    b1/b2 biases and the LN2 gamma are all folded into W1/W2 fix matmuls on
    the PE (relu(A2*x) = A2*relu(x) since A2>0), with the A2 column scale
    applied once at the output stage.
Everything is feature-major ("transposed") so every matmul has a natural lhsT.
"""

import numpy as np
import ml_dtypes

import concourse.bass as bass
import concourse.bacc as bacc
import concourse.mybir as mybir
import concourse.tile as tile
from concourse.bass_utils import run_bass_kernel_spmd

F32 = mybir.dt.float32
BF16 = mybir.dt.bfloat16
AF = mybir.ActivationFunctionType
ALU = mybir.AluOpType

B, T, D, H = 2, 2048, 1024, 16
HS = D // H          # 64
DFF = 4 * D          # 4096
EPS = 1e-5
NC_ = 8              # cores
BT = B * T           # 4096 flat tokens
TOK = BT // NC_      # 512 tokens per core
NBLK = BT // 512     # 8 token blocks
NDC = D // 128       # 8 d-chunks
NH_LOC = H // NC_    # 2 heads per core
NKT = T // 128       # 16 key tiles per batch
NHT = DFF // 128     # 32 hidden tiles
MASKB = -30.0        # additive pre-exp mask bias


def _build_nc():
    nc = bacc.Bacc(num_devices=NC_)

    xt_d = nc.dram_tensor("xt", [NBLK, 128, NDC, 512], BF16, kind="ExternalInput")
    xtloc_d = nc.dram_tensor("xt_loc", [128, NDC, 512], F32, kind="ExternalInput")
    wqkv_d = nc.dram_tensor("wqkv", [D, 384], BF16, kind="ExternalInput")
    wsb2_d = nc.dram_tensor("wsb2", [2, 384], F32, kind="ExternalInput")
    wsr_d = nc.dram_tensor("ws_row", [1, 384], F32, kind="ExternalInput")
    wbr_d = nc.dram_tensor("wb_row", [1, 384], F32, kind="ExternalInput")
    wproj_d = nc.dram_tensor("wproj", [128, NDC, D], BF16, kind="ExternalInput")
    w1_d = nc.dram_tensor("w1", [NHT, 128, NDC, 128], BF16, kind="ExternalInput")
    w1fix_d = nc.dram_tensor("w1fix", [2, DFF], F32, kind="ExternalInput")
    w2_d = nc.dram_tensor("w2", [NDC, 128, NHT, 128], BF16, kind="ExternalInput")
    bproj_d = nc.dram_tensor("bproj_pp", [128, 8], F32, kind="ExternalInput")
    b2p_d = nc.dram_tensor("b2p_pp", [128, 8], F32, kind="ExternalInput")
    b1ln_d = nc.dram_tensor("b1ln_pp", [128, 8], F32, kind="ExternalInput")
    g1diag_d = nc.dram_tensor("g1diag", [128, NDC, 128], F32, kind="ExternalInput")
    g1row_d = nc.dram_tensor("g1row", [1, D], F32, kind="ExternalInput")
    g2diag_d = nc.dram_tensor("g2diag", [128, NDC, 128], BF16, kind="ExternalInput")
    g2row_d = nc.dram_tensor("g2row", [1, D], F32, kind="ExternalInput")
    maskb_d = nc.dram_tensor("maskb", [128, 128], BF16, kind="ExternalInput")
    identb_d = nc.dram_tensor("identb", [128, 128], BF16, kind="ExternalInput")
    ident_d = nc.dram_tensor("ident", [128, 128], F32, kind="ExternalInput")
    onesd_d = nc.dram_tensor("ones_d", [128, 1], F32, kind="ExternalInput")    # 1/D
    onesdb_d = nc.dram_tensor("ones_db", [128, 1], BF16, kind="ExternalInput")  # 1/D bf16
    ones1_d = nc.dram_tensor("ones_1", [1, 128], F32, kind="ExternalInput")
    ones1b_d = nc.dram_tensor("ones_1b", [1, 128], BF16, kind="ExternalInput")
    out_d = nc.dram_tensor("outT", [D, TOK], F32, kind="ExternalOutput")

    with tile.TileContext(nc) as tc:
        with tc.tile_pool(name="const", bufs=1) as cst, \
             tc.tile_pool(name="dram", bufs=1, space="DRAM") as dpool:
            def cload(shape, dram_ap, dtype=F32):
                t = cst.tile(shape, dtype, name=f"c{len(nc.m.functions[0].allocations)}")
                nc.gpsimd.dma_start(t[:], dram_ap)
                return t

            # dummy collective FIRST: absorbs the ~30us first-collective
            # firmware barrier while real work proceeds (result unused)
            dum_in = dpool.tile([1, 8], BF16, name="dum_i")
            dum_out = dpool.tile([8, 8], BF16, name="dum_o")

            # ---- HAM warmup: keep PE busy while constants stream in ----
            with tc.tile_pool(name="warm", bufs=1) as wp, \
                 tc.tile_pool(name="warm_ps", bufs=1, space="PSUM") as wps:
                wt_ = wp.tile([128, 512], BF16)
                nc.vector.memset(wt_[:], 0.001)
                nc.sync.dma_start(dum_in[:], wt_[0:1, 0:8])
                nc.gpsimd.collective_compute(
                    "AllGather", ALU.bypass,
                    replica_groups=[list(range(NC_))],
                    ins=[dum_in.opt()], outs=[dum_out.opt()])
                wpt = wps.tile([128, 512], F32)
                for _ in range(16):
                    nc.tensor.matmul(wpt[:], wt_[:, 0:128], wt_[:], start=True, stop=True)

            wqkv_sb = cload([128, NDC, 384], wqkv_d[:, :].rearrange("(a p) m -> p a m", p=128), BF16)
            maskb_sb = cload([128, 128], maskb_d[:, :], BF16)
            identb_sb = cload([128, 128], identb_d[:, :], BF16)
            ident_sb = cload([128, 128], ident_d[:, :])
            onesd_sb = cload([128, 1], onesd_d[:, :])
            onesdb_sb = cload([128, 1], onesdb_d[:, :], BF16)
            ones1_sb = cload([1, 128], ones1_d[:, :])
            wsb2_sb = cload([2, 384], wsb2_d[:, :])
            wsr_sb = cload([1, 384], wsr_d[:, :])
            wbr_sb = cload([1, 384], wbr_d[:, :])
            ones1b_sb = cload([1, 128], ones1b_d[:, :], BF16)
            g1diag_sb = cload([128, NDC, 128], g1diag_d[:, :, :])
            g1row_sb = cload([1, D], g1row_d[:, :])
            g2diag_sb = cload([128, NDC, 128], g2diag_d[:, :, :], BF16)
            g2row_sb = cload([1, D], g2row_d[:, :])
            bproj_sb = cload([128, 8], bproj_d[:, :])
            b2p_sb = cload([128, 8], b2p_d[:, :])
            b1ln_sb = cload([128, 8], b1ln_d[:, :])
            w1fix_sb = cload([2, DFF], w1fix_d[:, :])

            xw = cst.tile([128, NDC, 512], F32)      # x_loc -> xn_loc -> x1_loc
            x1b = cst.tile([128, NDC, 512], BF16)    # bf16 copy of x1 for FFN
            stats2 = cst.tile([2, 512], F32)         # LN2 rows (-mu2; sd2)
            a2bc = cst.tile([128, 512], F32)         # broadcast A2 row
            Gr_sb = cst.tile([2, NC_, 512], F32)     # gathered (-mu, sd) rows per rank
            Ar_sb = cst.tile([1, NC_, 512], F32)     # gathered 1/sd rows per rank
            GT_sb = cst.tile([128, 4, 24], F32)      # transposed stats (A_k columns)
            hT = cst.tile([128, NHT, 512], BF16)     # FFN hidden
            attn_loc = cst.tile([128, NDC, 512], BF16)

            nc.sync.dma_start(xw[:], xtloc_d[:, :, :])

            # ===== local LN1 stats for my 512 tokens + AllGather =====
            with (
                tc.tile_pool(name="st_sm", bufs=1) as stp,
                tc.tile_pool(name="st_ps", bufs=1, space="PSUM") as stps,
            ):
                s1p = stps.tile([1, 512], F32)
                s2p = stps.tile([1, 512], F32)
                for a in range(NDC):
                    sq = stp.tile([128, 512], BF16, tag="sq")
                    if a % 2 == 0:
                        nc.scalar.square(sq[:], xw[:, a, :])
                    else:
                        nc.vector.tensor_tensor(sq[:], xw[:, a, :], xw[:, a, :], ALU.mult)
                    nc.tensor.matmul(s1p[:], onesd_sb[:], xw[:, a, :],
                                     start=(a == 0), stop=(a == NDC - 1))
                    nc.tensor.matmul(s2p[:], onesdb_sb[:], sq[:],
                                     start=(a == 0), stop=(a == NDC - 1))
                nmu_l = stp.tile([1, 512], F32)   # -mu
                sd_l = stp.tile([1, 512], F32)    # sd
                arow_l = stp.tile([1, 512], F32)  # 1/sd
                msq = stp.tile([1, 512], F32)
                nc.scalar.square(msq[:], s1p[:])
                veps = stp.tile([1, 512], F32)
                nc.vector.scalar_tensor_tensor(veps[:], s2p[:], EPS, msq[:],
                                               ALU.add, ALU.subtract)
                nc.scalar.sqrt(sd_l[:], veps[:])
                nc.vector.reciprocal(arow_l[:], sd_l[:])
                nc.vector.tensor_scalar_mul(nmu_l[:], s1p[:], -1.0)

                ag_in = dpool.tile([3, 512], F32, name="ag_in")
                ag_out = dpool.tile([24, 512], F32, name="ag_out")
                nc.sync.dma_start(ag_in[0:1], nmu_l[:])
                nc.sync.dma_start(ag_in[1:2], sd_l[:])
                nc.sync.dma_start(ag_in[2:3], arow_l[:])
                nc.gpsimd.collective_compute(
                    "AllGather", ALU.bypass,
                    replica_groups=[list(range(NC_))],
                    ins=[ag_in.opt()], outs=[ag_out.opt()])
                ag_r = ag_out[:, :].rearrange("(r s) n -> s r n", s=3)
                nc.sync.dma_start(Gr_sb[:], ag_r[0:2])
                nc.sync.dma_start(Ar_sb[:], ag_r[2:3])
                G24 = stp.tile([24, 512], F32)
                nc.sync.dma_start(G24[:], ag_out[:, :])
                for c4 in range(4):
                    gtp = stps.tile([128, 24], F32, tag="gtp", bufs=2)
                    nc.tensor.transpose(gtp[:], G24[:, 128 * c4:128 * (c4 + 1)],
                                        ident_sb[0:24, 0:24])
                    nc.vector.tensor_copy(GT_sb[:, c4, :], gtp[:])

                # ---- LN1 apply on local slice: xn = Abc*(g1*x + g1*(-mu)) + b1ln ----
                abl = stps.tile([128, 512], F32)
                nc.tensor.matmul(abl[:], ones1_sb[:], arow_l[:], start=True, stop=True)
                abl_sb = stp.tile([128, 512], F32)
                nc.scalar.copy(abl_sb[:], abl[:])
                for a in range(NDC):
                    pa = stps.tile([128, 512], F32, tag="pa", bufs=2)
                    nc.tensor.matmul(pa[:], g1diag_sb[:, a, :], xw[:, a, :],
                                     start=True, stop=False)
                    nc.tensor.matmul(pa[:], g1row_sb[:, 128 * a:128 * (a + 1)],
                                     nmu_l[:], start=False, stop=True)
                    t1 = stp.tile([128, 512], F32, tag="t1", bufs=2)
                    nc.vector.tensor_tensor(t1[:], pa[:], abl_sb[:], ALU.mult)
                    nc.scalar.activation(xw[:, a, :], t1[:], AF.Identity,
                                         bias=b1ln_sb[:, a:a + 1])

            # ===== big attention scope =====
            with tc.tile_pool(name="attn_scope", bufs=1) as big:
                Q2 = big.tile([128, BT], BF16)
                K2 = big.tile([128, BT], BF16)
                VK = big.tile([128, BT], BF16)   # k-major V, heads side by side
                attnT = big.tile([128, BT], BF16)

                # ===== Phase A: raw QKV + PE-folded LN fixups =====
                with (
                    tc.tile_pool(name="sweep", bufs=2) as swp,
                    tc.tile_pool(name="sweep_ps", bufs=2, space="PSUM") as swps,
                    tc.tile_pool(name="fix_ps", bufs=1, space="PSUM") as fxps,
                ):
                    for tb in range(NBLK):
                        xblk = swp.tile([128, NDC, 512], BF16, tag="xblk")
                        nc.sync.dma_start(xblk[:], xt_d[tb])
                        # rank/col-offset of this block's two 256-token halves
                        halves = [(2 * (tb % 4) + h, 256 * (tb // 4)) for h in range(2)]
                        qkps = []
                        for j in range(3):
                            ps = swps.tile([128, 512], F32, tag="qkvps", bufs=5)
                            for a in range(NDC):
                                nc.tensor.matmul(ps[:], wqkv_sb[:, a, 128 * j:128 * (j + 1)],
                                                 xblk[:, a, :],
                                                 start=(a == 0), stop=(a == NDC - 1))
                            for h, (r, c0) in enumerate(halves):
                                nc.tensor.matmul(
                                    ps[:, 256 * h:256 * (h + 1)],
                                    wsb2_sb[:, 128 * j:128 * (j + 1)],
                                    Gr_sb[:, r, c0:c0 + 256],
                                    start=False, stop=(h == 1), skip_group_check=True)
                            qkps.append(ps)
                        bap = fxps.tile([128, 512], F32, tag="bap")
                        for h, (r, c0) in enumerate(halves):
                            nc.tensor.matmul(bap[:, 256 * h:256 * (h + 1)], ones1_sb[:],
                                             Ar_sb[:, r, c0:c0 + 256],
                                             start=(h == 0), stop=(h == 1),
                                             skip_group_check=True)
                        ba_sb = swp.tile([128, 512], F32, tag="ba_sb")
                        nc.scalar.copy(ba_sb[:], bap[:])
                        vsb = swp.tile([128, 512], F32, tag="vsb")
                        nc.scalar.copy(vsb[:], qkps[2][:])
                        ts_ = slice(512 * tb, 512 * (tb + 1))
                        nc.scalar.copy(K2[:, ts_], qkps[1][:])
                        nc.vector.tensor_tensor(Q2[:, ts_], qkps[0][:], ba_sb[:], ALU.mult)
                        for q in range(4):
                            vtp = fxps.tile([128, 128], F32, tag="vtp", bufs=2)
                            nc.tensor.transpose(vtp[:], vsb[:, 128 * q:128 * (q + 1)],
                                                ident_sb[:])
                            nc.vector.tensor_copy(
                                VK[:, 512 * tb + 128 * q:512 * tb + 128 * (q + 1)],
                                vtp[:])

                # ===== Phase B: attention per (batch, key-tile), A2A per batch =====
                with (
                    tc.tile_pool(name="se_pool", bufs=2) as sep,
                    tc.tile_pool(name="attn_sm", bufs=2) as asm,
                    tc.tile_pool(name="st_ps2", bufs=1, space="PSUM") as sps2,
                    tc.tile_pool(name="ap_ps", bufs=1, space="PSUM") as apps,
                ):
                    for b in range(B):
                        ap_tiles = [apps.tile([128, 512], F32, name=f"app{b}_{qb}",
                                              tag=f"ap{qb}", bufs=1)
                                    for qb in range(4)]
                        se_tiles, vc_tiles, st_tiles, ck_tiles = {}, {}, {}, {}

                        def ascale(kt):
                            return GT_sb[:, 2 * b + (kt % 2), 3 * (kt // 2) + 2:
                                         3 * (kt // 2) + 3]

                        def st_mms(kt, hh, c):
                            # chunk c covers region cols [1024c, min(1024(c+1), W))
                            qb0 = kt // 4
                            co = 128 * (kt % 4)
                            W = 2048 - 512 * qb0
                            q0 = b * T + 512 * qb0
                            stp_ = sps2.tile([128, 1024], F32, tag=f"st{hh}", bufs=1)
                            st_tiles[(kt, hh, c)] = stp_
                            hsl = slice(64 * hh, 64 * hh + 64)
                            lo = max(co, 1024 * c)
                            hi = min(W, 1024 * (c + 1))
                            g = lo
                            while g < hi:
                                g2 = min(hi, (g // 512 + 1) * 512)
                                nc.tensor.matmul(
                                    stp_[:, g - 1024 * c:g2 - 1024 * c],
                                    K2[hsl, b * T + 128 * kt:b * T + 128 * (kt + 1)],
                                    Q2[hsl, q0 + g:q0 + g2],
                                    start=True, stop=False,
                                    skip_group_check=True)
                                g = g2
                            if c == 0:
                                nc.tensor.matmul(
                                    stp_[:, co:co + 128], identb_sb[:],
                                    maskb_sb[:, :],
                                    start=False, stop=True, skip_group_check=True)

                        def exp_chunk(kt, hh, c, nchunk):
                            qb0 = kt // 4
                            co = 128 * (kt % 4)
                            W = 2048 - 512 * qb0
                            stp_ = st_tiles.pop((kt, hh, c))
                            lo = max(co, 1024 * c)
                            hi = min(W, 1024 * (c + 1))
                            if c == 0:
                                se = sep.tile([128, 2048], BF16,
                                              name=f"se_{b}_{kt}_{hh}",
                                              tag=f"se{hh}", bufs=3)
                                se_tiles[(kt, hh)] = se
                                ck = asm.tile([128, 2], F32, tag=f"ck{hh}", bufs=3)
                                ck_tiles[(kt, hh)] = ck
                            se = se_tiles[(kt, hh)]
                            ck = ck_tiles[(kt, hh)]
                            nc.scalar.activation(
                                se[:, lo:hi], stp_[:, lo - 1024 * c:hi - 1024 * c],
                                AF.Exp, scale=ascale(kt),
                                accum_out=ck[:, c:c + 1])
                            if c == nchunk - 1:
                                if nchunk == 2:
                                    cks = asm.tile([128, 1], F32, tag=f"cks{hh}", bufs=3)
                                    nc.vector.tensor_tensor(cks[:], ck[:, 0:1],
                                                            ck[:, 1:2], ALU.add)
                                    cks_ap = cks[:]
                                else:
                                    cks_ap = ck[:, 0:1]
                                rk = asm.tile([128, 1], F32, tag=f"rk{hh}", bufs=3)
                                nc.vector.reciprocal(rk[:], cks_ap)
                                ark = asm.tile([128, 1], F32, tag=f"ark{hh}", bufs=3)
                                nc.vector.tensor_tensor(ark[:], rk[:], ascale(kt),
                                                        ALU.mult)
                                vc = asm.tile([128, 64], BF16, tag=f"vc{hh}", bufs=3)
                                ktf = b * NKT + kt
                                nc.vector.tensor_scalar_mul(
                                    vc[:],
                                    VK[:, 128 * ktf + 64 * hh:128 * ktf + 64 * hh + 64],
                                    ark[:])
                                vc_tiles[(kt, hh)] = vc

                        def emit_at(kt):
                            qb0 = kt // 4
                            co = 128 * (kt % 4)
                            for qb in range(qb0, 4):
                                off = 512 * (qb - qb0)
                                n0 = co if qb == qb0 else 0
                                last = (kt == 4 * (qb + 1) - 1)
                                for hh in range(NH_LOC):
                                    nc.tensor.matmul(
                                        ap_tiles[qb][64 * hh:64 * hh + 64, n0:512],
                                        vc_tiles[(kt, hh)][:],
                                        se_tiles[(kt, hh)][:, off + n0:off + 512],
                                        start=(kt == 0), stop=last,
                                        tile_position=(0, 64 * hh),
                                        skip_group_check=True)
                                if last:
                                    nc.vector.tensor_copy(
                                        attnT[:, b * T + 512 * qb:b * T + 512 * (qb + 1)],
                                        ap_tiles[qb][:])

                        for kt in range(NKT + 1):
                            if kt < NKT:
                                W = 2048 - 512 * (kt // 4)
                                nchunk = (W + 1023) // 1024
                                for c in range(nchunk):
                                    st_mms(kt, 0, c)
                                    st_mms(kt, 1, c)
                                    exp_chunk(kt, 0, c, nchunk)
                                    exp_chunk(kt, 1, c, nchunk)
                            if kt >= 1:
                                emit_at(kt - 1)

                        a2a_in = dpool.tile([8, 128, 256], BF16, name=f"a2ai{b}", tag=f"a2ai{b}")
                        a2a_out = dpool.tile([8, 128, 256], BF16, name=f"a2ao{b}", tag=f"a2ao{b}")
                        for j in range(8):
                            nc.sync.dma_start(a2a_in[j],
                                              attnT[:, b * T + 256 * j:b * T + 256 * (j + 1)])
                        nc.gpsimd.collective_compute(
                            "AllToAll", ALU.bypass,
                            replica_groups=[list(range(NC_))],
                            ins=[a2a_in.opt()], outs=[a2a_out.opt()])
                        for s in range(8):
                            nc.sync.dma_start(attn_loc[:, s, 256 * b:256 * (b + 1)], a2a_out[s])

            # ===== Phase D: projection + residual + LN2 stats (PE-folded apply) =====
            with (
                tc.tile_pool(name="proj_sm", bufs=2) as pjm,
                tc.tile_pool(name="proj_ps", bufs=2, space="PSUM") as pjps,
            ):
                for dt in range(NDC):
                    wpt = pjm.tile([128, NDC, 128], BF16, tag="wp", bufs=3)
                    nc.sync.dma_start(wpt[:], wproj_d[:, :, 128 * dt:128 * (dt + 1)])
                    pp = pjps.tile([128, 512], F32, tag="pp")
                    for a in range(NDC):
                        nc.tensor.matmul(pp[:], wpt[:, a, :],
                                         attn_loc[:, a, :],
                                         start=(a == 0), stop=(a == NDC - 1))
                    nc.vector.scalar_tensor_tensor(
                        xw[:, dt, :], pp[:], bproj_sb[:, dt:dt + 1], xw[:, dt, :],
                        ALU.add, ALU.add)
                    nc.scalar.copy(x1b[:, dt, :], xw[:, dt, :])
                # LN2 stats on x1
                s1p = pjps.tile([1, 512], F32, tag="s1b", bufs=1)
                s2p = pjps.tile([1, 512], F32, tag="s2b", bufs=1)
                for a in range(NDC):
                    sq = pjm.tile([128, 512], BF16, tag="sq2")
                    nc.vector.tensor_tensor(sq[:], x1b[:, a, :], x1b[:, a, :], ALU.mult)
                    nc.tensor.matmul(s1p[:], onesd_sb[:], xw[:, a, :],
                                     start=(a == 0), stop=(a == NDC - 1))
                    nc.tensor.matmul(s2p[:], onesdb_sb[:], sq[:],
                                     start=(a == 0), stop=(a == NDC - 1))
                msq = pjm.tile([1, 512], F32, tag="msq2", bufs=1)
                nc.scalar.square(msq[:], s1p[:])
                veps = pjm.tile([1, 512], F32, tag="veps2", bufs=1)
                nc.vector.scalar_tensor_tensor(veps[:], s2p[:], EPS, msq[:],
                                               ALU.add, ALU.subtract)
                sd2_l = pjm.tile([1, 512], F32, tag="sd2l", bufs=1)
                nc.scalar.sqrt(sd2_l[:], veps[:])
                a2row = pjm.tile([1, 512], F32, tag="a2row", bufs=1)
                nc.vector.reciprocal(a2row[:], sd2_l[:])
                nc.vector.tensor_scalar_mul(stats2[0:1, :], s1p[:], -1.0)
                nc.sync.dma_start(stats2[1:2, :], sd2_l[:])
                a2p = pjps.tile([128, 512], F32, tag="a2p", bufs=1)
                nc.tensor.matmul(a2p[:], ones1_sb[:], a2row[:], start=True, stop=True)
                nc.scalar.copy(a2bc[:], a2p[:])

            # ===== Phase E: FFN (token-sharded, streamed weights) =====
            with (
                tc.tile_pool(name="ffn_w1", bufs=3) as w1p,
                tc.tile_pool(name="ffn_w2", bufs=2) as w2p,
                tc.tile_pool(name="ffn_sm", bufs=2) as fsm,
                tc.tile_pool(name="ffn_ps", bufs=2, space="PSUM") as fps,
            ):
                for ht in range(NHT):
                    w1t = w1p.tile([128, NDC, 128], BF16, tag="w1", bufs=4)
                    nc.sync.dma_start(w1t[:], w1_d[ht])
                    hp = fps.tile([128, 512], F32, tag="hp")
                    for a in range(NDC):
                        nc.tensor.matmul(hp[:], w1t[:, a, :], x1b[:, a, :],
                                         start=(a == 0), stop=False)
                    nc.tensor.matmul(hp[:], w1fix_sb[:, 128 * ht:128 * (ht + 1)],
                                     stats2[:, :], start=False, stop=True)
                    nc.scalar.activation(hT[:, ht, :], hp[:], AF.Relu)
                for dt in range(NDC):
                    w2t = w2p.tile([128, NHT, 128], BF16, tag="w2")
                    nc.sync.dma_start(w2t[:], w2_d[dt])
                    fp_ = fps.tile([128, 512], F32, tag="fp")
                    for a2_ in range(NHT):
                        nc.tensor.matmul(fp_[:], w2t[:, a2_, :], hT[:, a2_, :],
                                         start=(a2_ == 0), stop=False)
                    nc.tensor.matmul(fp_[:], g2diag_sb[:, dt, :], x1b[:, dt, :],
                                     start=False, stop=False)
                    nc.tensor.matmul(fp_[:], g2row_sb[:, 128 * dt:128 * (dt + 1)],
                                     stats2[0:1, :], start=False, stop=True)
                    t1 = fsm.tile([128, 512], F32, tag="t1f")
                    nc.vector.tensor_tensor(t1[:], fp_[:], a2bc[:], ALU.mult)
                    ot = fsm.tile([128, 512], F32, tag="ot")
                    nc.scalar.activation(ot[:], t1[:], AF.Identity,
                                         bias=b2p_sb[:, dt:dt + 1])
                    nc.sync.dma_start(
                        out_d[:, :].rearrange("(a p) n -> p a n", p=128)[:, dt, :], ot[:])
    nc.compile()
    return nc


_NC_CACHE = None


def _get_nc():
    global _NC_CACHE
    if _NC_CACHE is None:
        _NC_CACHE = _build_nc()
    return _NC_CACHE


def make_in_maps(inputs):
    x = np.asarray(inputs["x"], np.float32)
    Wq = np.asarray(inputs["Wq"], np.float32)
    Wk = np.asarray(inputs["Wk"], np.float32)
    Wv = np.asarray(inputs["Wv"], np.float32)
    Wproj = np.ascontiguousarray(np.asarray(inputs["Wproj"], np.float32))
    bproj = np.asarray(inputs["bproj"], np.float32)
    W1 = np.ascontiguousarray(np.asarray(inputs["W1"], np.float32))
    b1 = np.asarray(inputs["b1"], np.float32)
    W2 = np.ascontiguousarray(np.asarray(inputs["W2"], np.float32))
    b2 = np.asarray(inputs["b2"], np.float32)
    g1 = np.asarray(inputs["ln1_g"], np.float32)
    bl1 = np.asarray(inputs["ln1_b"], np.float32)
    g2 = np.asarray(inputs["ln2_g"], np.float32)
    bl2 = np.asarray(inputs["ln2_b"], np.float32)

    s = np.float32(D ** -0.5)
    x_flat = x.reshape(BT, D)
    xt = np.ascontiguousarray(
        x_flat.reshape(NBLK, 512, NDC, 128).transpose(0, 3, 2, 1)
    ).astype(ml_dtypes.bfloat16)

    # additive causal mask bias for the diagonal 128x128 window:
    # key i (partition), query col j: 0 if j >= i else MASKB
    jj = np.arange(128)[None, :]
    ii = np.arange(128)[:, None]
    maskb = np.where(jj >= ii, 0.0, MASKB).astype(np.float32)

    W1g = W1 * g2[:, None]
    w1fix = np.stack([W1g.sum(0), bl2 @ W1 + b1]).astype(np.float32)

    common = {
        "xt": xt,
        "wproj": np.ascontiguousarray(Wproj.reshape(NDC, 128, D).transpose(1, 0, 2)).astype(ml_dtypes.bfloat16),
        "w1": np.ascontiguousarray(W1g.reshape(NDC, 128, NHT, 128).transpose(2, 1, 0, 3)).astype(ml_dtypes.bfloat16),
        "w1fix": w1fix,
        "w2": np.ascontiguousarray(W2.reshape(NHT, 128, NDC, 128).transpose(2, 1, 0, 3)).astype(ml_dtypes.bfloat16),
        "bproj_pp": np.ascontiguousarray(bproj.reshape(8, 128).T),
        "b2p_pp": np.ascontiguousarray((bl2 + b2).reshape(8, 128).T),
        "b1ln_pp": np.ascontiguousarray(bl1.reshape(8, 128).T),
        "g1diag": np.ascontiguousarray(
            (np.eye(128)[None] * g1.reshape(NDC, 1, 128)).transpose(1, 0, 2)).astype(np.float32),
        "g1row": g1.reshape(1, D),
        "g2diag": np.ascontiguousarray(
            (np.eye(128)[None] * g2.reshape(NDC, 1, 128)).transpose(1, 0, 2)).astype(ml_dtypes.bfloat16),
        "g2row": g2.reshape(1, D),
        "maskb": maskb.astype(ml_dtypes.bfloat16),
        "identb": np.eye(128, dtype=np.float32).astype(ml_dtypes.bfloat16),
        "ident": np.eye(128, dtype=np.float32),
        "ones_d": np.full((128, 1), 1.0 / D, np.float32),
        "ones_db": np.full((128, 1), 1.0 / D, ml_dtypes.bfloat16),
        "ones_1": np.ones((1, 128), np.float32),
        "ones_1b": np.ones((1, 128), ml_dtypes.bfloat16),
    }

    in_maps = []
    for c in range(NC_):
        h0 = NH_LOC * c
        Wq_cat = np.concatenate([Wq[h0 + i] for i in range(NH_LOC)], 1)  # [D,128]
        Wk_cat = np.concatenate([Wk[h0 + i] for i in range(NH_LOC)], 1)
        Wv_cat = np.concatenate([Wv[h0 + i] for i in range(NH_LOC)], 1)
        Wq_eff = g1[:, None] * Wq_cat * s
        Wk_eff = g1[:, None] * Wk_cat
        Wv_eff = g1[:, None] * Wv_cat
        wqkv = np.ascontiguousarray(np.concatenate([Wq_eff, Wk_eff, Wv_eff], 1)).astype(ml_dtypes.bfloat16)
        wsums = np.concatenate([Wq_eff.sum(0), Wk_eff.sum(0), Wv_eff.sum(0)])
        wbias = np.concatenate([bl1 @ (Wq_cat * s), bl1 @ Wk_cat, bl1 @ Wv_cat])
        m = dict(common)
        xl = np.concatenate([x[0, 256 * c:256 * (c + 1)],
                             x[1, 256 * c:256 * (c + 1)]], axis=0)  # [512, D]
        m["xt_loc"] = np.ascontiguousarray(xl.reshape(512, NDC, 128).transpose(2, 1, 0))
        m["wqkv"] = wqkv
        m["wsb2"] = np.ascontiguousarray(
            np.stack([wsums, wbias]).astype(np.float32))
        m["ws_row"] = np.ascontiguousarray(wsums.reshape(1, 384).astype(np.float32))
        m["wb_row"] = np.ascontiguousarray(wbias.reshape(1, 384).astype(np.float32))
        in_maps.append(m)
    return in_maps


def run(inputs, trace=False, trace_kwargs=None):
    nc = _get_nc()
    in_maps = make_in_maps(inputs)
    res = run_bass_kernel_spmd(nc, in_maps, core_ids=list(range(NC_)),
                               trace=trace, **(trace_kwargs or {}))
    out = np.empty((B, T, D), np.float32)
    for c in range(NC_):
        o = res.results[c]["outT"]
        out[0, 256 * c:256 * (c + 1)] = o[:, 0:256].T
        out[1, 256 * c:256 * (c + 1)] = o[:, 256:512].T
    return out, res


def kernel(**inputs) -> np.ndarray:
    out, _ = run(inputs, trace=False)
    return out
